# revision 37
# baseline (speedup 1.0000x reference)
"""Dual-stream multi-head attention on 8 Trainium2 NeuronCores (Bass/Tile).

Sharding: core c handles batch b = c//4 and head-group g = c%4 (4 of 16 heads).
Each core computes QKV projections (per-stream weights), RoPE, joint attention
over both streams, and a partial output projection (its heads' rows of wo).
The host sums the 4 per-core partials of each batch (bf16 partials, fp32
accumulate), transposes, and adds the output bias.

Layout: fully transposed on-chip (features on partitions, tokens on the free
dim).  Scores are computed as S^T = k_rope @ q_rope^T so the PV matmul
consumes exp(S^T) with v in natural [token, dh] layout.

Differences vs the previous revision (targets ~90%+ PE occupancy):
- per-chunk weight tiles + DMA issue in consumption order on the Activation
  HWDGE queue (x/output tiles on the SP queue) — kills the 31+15us startup
  and stream-switch stalls where PE waited on monolithic slab DMAs.
- v-projection reuses the resident x chunk tiles as stationary operands
  (no second, strided reload of x: -8.4MB DMA per core).
- score psums are written in 2-bank pairs and exp'd with ONE activation
  instruction over 1024 free elements (halves Act instruction count; Act was
  the attention-phase critical path).
- exp accumulation (softmax denominators) in bf16 on the DVE (2-byte SBUF
  operands hit the DVE 4x mode); the final cross-partition reduction is a
  bf16 ones-matmul (the old fp32 ones-matmul ran at 1/4 PE speed).
- output projection of query-tile qt is interleaved 2-matmuls-per-pair-step
  into the attention of qt+1 so PE fills its exp-wait gaps; output written as
  bf16 quads, one DMA per 4 output-dim chunks (halves out DMA, amortizes the
  ~630ns/DMA HWDGE fixed cost).
"""

import sys
import numpy as np

sys.path.insert(0, "/opt/trn_rl_repo")

import ml_dtypes
import concourse.bass as bass
import concourse.mybir as mybir
import concourse.tile as tile
from concourse.bass_utils import run_bass_kernel_spmd
from contextlib import ExitStack

B, N1, N2, D, H = 2, 1024, 1024, 2048, 16
T = N1 + N2              # 2048 tokens (both streams, concatenated)
DH = D // H              # 128
HPC = 4                  # heads per core
NKC = D // 128           # 16 contraction chunks
NTT = T // 512           # 4 512-token tiles
SCALE = DH ** -0.5
N_CORES = 8

BF = mybir.dt.bfloat16
F32 = mybir.dt.float32
bf16 = ml_dtypes.bfloat16
AF = mybir.ActivationFunctionType
ALU = mybir.AluOpType

_BUILT = {}
SWAP_MASK = [i ^ 1 for i in range(32)]


def build_program(repeats=1, phases="ABCD"):
    global _BUILT
    key = (repeats, phases)
    if key in _BUILT:
        return _BUILT[key]

    nc = bass.Bass()

    # all big tensors are laid out partition-major on the host so every DMA
    # reads/writes contiguous bytes per partition (the "k p f -> p k f"
    # rearrange pattern only reaches ~180 GB/s; these hit full rate)
    xT_d = nc.dram_tensor("xT", [128, NTT, NKC, 512], BF, kind="ExternalInput")
    wq_d = nc.dram_tensor("wq", [128, 2, NKC, HPC * DH], BF, kind="ExternalInput")
    wk_d = nc.dram_tensor("wk", [128, 2, NKC, HPC * DH], BF, kind="ExternalInput")
    wv_d = nc.dram_tensor("wv", [128, 2, NKC, HPC * DH], BF, kind="ExternalInput")
    wo_d = nc.dram_tensor("wo", [2, HPC, 128, D], BF, kind="ExternalInput")
    bias_d = nc.dram_tensor("bias_qk", [128, 17], F32, kind="ExternalInput")
    bvb_d = nc.dram_tensor("bvb", [128, 2, 2, 512], BF, kind="ExternalInput")
    cos_d = nc.dram_tensor("cosT", [128, T], BF, kind="ExternalInput")
    sin_d = nc.dram_tensor("sinT", [128, T], BF, kind="ExternalInput")
    out_d = nc.dram_tensor("outT", [128, NTT, NKC, 512], BF, kind="ExternalOutput")

    with tile.TileContext(nc) as tc:
        for _ in range(repeats):
            _emit(tc, nc, xT_d, wq_d, wk_d, wv_d, wo_d, bias_d, bvb_d, cos_d,
                  sin_d, out_d, phases=phases)

    _split_dma_waits(nc)
    _BUILT[key] = nc
    return nc


def _split_dma_waits(nc):
    """This walrus build's 64-byte instruction encoding holds exactly one sync
    wait; peel extras into standalone EventSemaphore waits on the same
    engine immediately before the instruction."""
    wid = 0
    fn = nc.m.functions[0]
    for blk in fn.blocks:
        insts = blk.instructions
        out = []
        changed = False
        for inst in insts:
            si = inst.sync_info
            if si is not None and len(si.on_wait) > 1:
                waits = list(si.on_wait)
                for w in waits[:-1]:
                    pre = mybir.InstEventSemaphore(
                        name=f"WSPLIT-{wid}", ins=[], outs=[])
                    wid += 1
                    pre.engine = inst.engine
                    pre.sync_info = mybir.SyncInfo(on_wait=[w], on_update=[])
                    nc.register_instruction(pre, overwrite=True)
                    out.append(pre)
                inst.sync_info = mybir.SyncInfo(
                    on_wait=[waits[-1]], on_update=list(si.on_update))
                changed = True
            out.append(inst)
        if changed:
            blk.instructions = out


def _emit(tc, nc, xT_d, wq_d, wk_d, wv_d, wo_d, bias_d, bvb_d, cos_d, sin_d,
          out_d, phases="ABCD"):
    with ExitStack() as top:
        consts = top.enter_context(tc.tile_pool(name="consts", bufs=1))
        persist = top.enter_context(tc.tile_pool(name="persist", bufs=1))

        bias_t = consts.tile([128, 17], F32, name="bias_t", tag="bias_t")
        bvb_t = consts.tile([128, 2, 2, 512], BF, name="bvb_t", tag="bvb_t")
        ones_t = consts.tile([128, 128], BF, name="ones_t", tag="ones_t")
        nc.vector.memset(ones_t[:], 1.0)
        zero_t = consts.tile([128, 1], F32, name="zero_t", tag="zero_t")
        nc.vector.memset(zero_t[:], 0.0)
        cosT = consts.tile([128, T], BF, name="cosT_t", tag="cosT_t")
        sinT = consts.tile([128, T], BF, name="sinT_t", tag="sinT_t")

        # qk_rope[h]: plane 0 = q_rope, plane 1 = k_rope  (bf16, [128, 2, T])
        qk_rope = [persist.tile([128, 2, T], BF, name=f"qkr{h}", tag=f"qkr{h}")
                   for h in range(HPC)]
        # v pairs: v_pair[p] covers token chunks (2p, 2p+1), natural layout
        v_pair = [persist.tile([128, 2, HPC * DH], BF, name=f"vp{p}", tag=f"vp{p}")
                  for p in range(NKC // 2)]

        # ---------------- Phase A: q,k projections + RoPE ------------------
        # ---------------- Phase B: v (natural layout) ----------------------
        with ExitStack() as ab:
            # per-stream full weight tiles [128, 16, 512] (tags carry s so
            # stream-1 DMAs prefetch during stream 0 without WAR waits); wv
            # reuses one tag (its s=1 DMA WAR-waits v(tt1)).  Stream-0 wq/wk
            # and the tt0 x tile are delivered as 2-chunk pieces interleaved
            # in consumption order so PE starts ~1us in.
            w_pool = ab.enter_context(tc.tile_pool(name="w", bufs=1))
            x_pool = ab.enter_context(tc.tile_pool(name="xs", bufs=2))
            x0_pool = ab.enter_context(tc.tile_pool(name="x0", bufs=1))
            sb_pool = ab.enter_context(tc.tile_pool(name="sb", bufs=2))
            t_pool = ab.enter_context(tc.tile_pool(name="tt", bufs=2))
            # one psum pool, 4 two-bank tags: qkp0/qkp1 alternate across
            # heads; rv0/rv1 are shared by the rotation psums (qk block) and
            # the v psums (v block) whose lifetimes interleave
            a_ps = ab.enter_context(tc.tile_pool(name="aps", bufs=1, space="PSUM"))

            HC = NKC // 2    # chunks per half-slab

            xt = {}          # tt -> (lo, hi) tiles [128, 8, 512]; tt=0: pieces
            x0p = []         # tt0 as 8 [128, 2, 512] piece tiles

            def xsl(tt, kc, toff=0, tn=512):
                if tt == 0:
                    return x0p[kc // 2][:, kc % 2, toff:toff + tn]
                return xt[tt][kc // HC][:, kc % HC, toff:toff + tn]

            def load_x(tt):
                # tt != 0 only (tt0 pieces are interleaved in the s==0 block)
                halves = []
                for i, nmi in ((0, "lo"), (1, "hi")):
                    t = x_pool.tile([128, HC, 512], BF, name=f"x{tt}{nmi}", tag=f"x{nmi}")
                    nc.sync.dma_start(t[:], xT_d[:, tt, i * HC:(i + 1) * HC, :])
                    halves.append(t)
                xt[tt] = halves

            def load_w(dram, s, tag, tag_s=True):
                t = w_pool.tile([128, NKC, HPC * DH], BF, name=f"{tag}{s}",
                                tag=f"{tag}{s}" if tag_s else tag)
                for i in range(2):
                    nc.sync.dma_start(t[:, i * HC:(i + 1) * HC, :],
                                      dram[:, s, i * HC:(i + 1) * HC, :])
                return t

            def emit_v(tt, s):
                for pl in range(2):
                    vp = tt * 2 + pl
                    vps = a_ps.tile([128, 2, 512], F32, name=f"vps{vp}", tag=f"rv{pl}")
                    for plane in range(2):
                        toff = pl * 256 + plane * 128
                        for kc in range(NKC):
                            nc.tensor.matmul(
                                vps[:, plane, :], xsl(tt, kc, toff, 128),
                                wv_t[s][:, kc, :],
                                start=(kc == 0), stop=(kc == NKC - 1))
                    # + bv (broadcast tile), psum -> sbuf on DVE
                    nc.vector.tensor_tensor(v_pair[vp][:], vps[:], bvb_t[:, s],
                                            ALU.add)

            def rope_tail(tt, s, h, qkp_or_sb):
                tsl = slice(tt * 512, (tt + 1) * 512)
                qkp = qkp_or_sb
                # sb2: plane 0 = q+bias, plane 1 = k+bias (contiguous)
                sb2 = sb_pool.tile([128, 2, 512], BF, name=f"sb2{tt}{h}", tag="sb2")
                nc.scalar.activation(sb2[:, 0, :], qkp[:, 0, :], AF.Identity,
                                     bias=bias_t[:, s * 8 + h:s * 8 + h + 1])
                nc.scalar.activation(sb2[:, 1, :], qkp[:, 1, :], AF.Identity,
                                     bias=bias_t[:, s * 8 + 4 + h:s * 8 + 4 + h + 1])
                # RoPE pair rotation as a partition pair-swap (2 SBUF->SBUF
                # DMAs on the otherwise DMA-free Act queue; the +/- sign
                # pattern is folded into the host-prepared sinT table).
                # Same-engine ordering after the sb2 acts, so no sync cost.
                sw = t_pool.tile([128, 2, 512], BF, name=f"sw{tt}{h}", tag="sw")
                nc.scalar.dma_start(sw[0:128:2, :, :], sb2[1:128:2, :, :])
                nc.scalar.dma_start(sw[1:128:2, :, :], sb2[0:128:2, :, :])
                t12 = t_pool.tile([128, 2, 512], BF, name=f"t12_{tt}{h}", tag="t12")
                nc.vector.tensor_tensor(t12[:, 0, :], sb2[:, 0, :], cosT[:, tsl], ALU.mult)
                nc.vector.tensor_tensor(t12[:, 1, :], sb2[:, 1, :], cosT[:, tsl], ALU.mult)
                t3 = t_pool.tile([128, 2, 512], BF, name=f"t3_{tt}{h}", tag="t3")
                nc.vector.tensor_tensor(t3[:, 0, :], sw[:, 0, :], sinT[:, tsl], ALU.mult)
                nc.vector.tensor_tensor(t3[:, 1, :], sw[:, 1, :], sinT[:, tsl], ALU.mult)
                nc.vector.tensor_tensor(qk_rope[h][:, :, tsl], t12[:], t3[:], ALU.add)

            def emit_qk(tt, s):
                for h in range(HPC):
                    hsl = slice(h * DH, (h + 1) * DH)
                    qkp = a_ps.tile([128, 2, 512], F32, name=f"qkp{tt}{h}",
                                    tag=f"qkp{h % 2}")
                    for kc in range(NKC):
                        nc.tensor.matmul(qkp[:, 0, :], wq_t[s][:, kc, hsl],
                                         xsl(tt, kc),
                                         start=(kc == 0), stop=(kc == NKC - 1))
                        nc.tensor.matmul(qkp[:, 1, :], wk_t[s][:, kc, hsl],
                                         xsl(tt, kc),
                                         start=(kc == 0), stop=(kc == NKC - 1))
                    rope_tail(tt, s, h, qkp)

            def emit_qk_pairs(tt, s):
                # chunk-major over head pairs: consumes each weight chunk at
                # 852ns vs the ~730ns/chunk DMA pipe delivery, so tt0's q/k
                # never outruns the interleaved wq/wk piece stream.
                for hp in range(2):
                    pair = (2 * hp, 2 * hp + 1)
                    qkps = [a_ps.tile([128, 2, 512], F32, name=f"qkp{tt}{h}",
                                       tag=f"qkp{h % 2}") for h in pair]
                    for kc in range(NKC):
                        for i, h in enumerate(pair):
                            hsl = slice(h * DH, (h + 1) * DH)
                            nc.tensor.matmul(qkps[i][:, 0, :], wq_t[s][:, kc, hsl],
                                             xsl(tt, kc),
                                             start=(kc == 0), stop=(kc == NKC - 1))
                            nc.tensor.matmul(qkps[i][:, 1, :], wk_t[s][:, kc, hsl],
                                             xsl(tt, kc),
                                             start=(kc == 0), stop=(kc == NKC - 1))
                    for i, h in enumerate(pair):
                        rope_tail(tt, s, h, qkps[i])

            # PE warmup: dummy matmuls from ~0.4us until the first v-chunk
            # DMAs land (~2.6us) keep the HAM activity window hot so real
            # matmuls start at full clock.
            wps = a_ps.tile([128, 512], F32, name="warm", tag="qkp0")
            for _ in range(20):
                nc.tensor.matmul(wps[:, 0:128], ones_t[:], ones_t[:],
                                 start=True, stop=True)

            wq_t, wk_t, wv_t = {}, {}, {}
            for s in range(2):
                if s == 0:
                    # ALL input DMAs ride ONE queue (SP) in exact global
                    # consumption order — the DMA pipe is a single shared
                    # ~350 GB/s resource, so cross-queue interleaving only
                    # scrambles the arrival order.  tt0 runs its
                    # v-projection FIRST (consumes 1 chunk per 4 matmuls =
                    # 852ns vs 730ns/chunk delivery) while wq/wk stream in
                    # behind wv; the Act queue carries no DMAs at all (a
                    # dma_start costs ~660ns on the issuing sequencer).
                    wv_t[0] = w_pool.tile([128, NKC, HPC * DH], BF, name="wv0",
                                          tag="wv")
                    for pc in range(8):
                        csl = slice(2 * pc, 2 * pc + 2)
                        t = x0_pool.tile([128, 2, 512], BF, name=f"x0p{pc}",
                                         tag=f"x0p{pc}")
                        nc.sync.dma_start(t[:], xT_d[:, 0, csl, :])
                        x0p.append(t)
                        nc.sync.dma_start(wv_t[0][:, csl, :], wv_d[:, 0, csl, :])
                    nc.sync.dma_start(bvb_t[:], bvb_d[:])
                    nc.sync.dma_start(bias_t[:], bias_d[:])
                    # wq/wk interleaved 2-chunk pieces in tt0's kc
                    # consumption order
                    wq_t[0] = w_pool.tile([128, NKC, HPC * DH], BF, name="wq0",
                                          tag="wq0")
                    wk_t[0] = w_pool.tile([128, NKC, HPC * DH], BF, name="wk0",
                                          tag="wk0")
                    for pc in range(8):
                        csl = slice(2 * pc, 2 * pc + 2)
                        nc.sync.dma_start(wq_t[0][:, csl, :], wq_d[:, 0, csl, :])
                        nc.sync.dma_start(wk_t[0][:, csl, :], wk_d[:, 0, csl, :])
                    nc.sync.dma_start(cosT[:], cos_d[:])
                    nc.sync.dma_start(sinT[:], sin_d[:])
                else:
                    wq_t[1] = load_w(wq_d, 1, "wq")
                    wk_t[1] = load_w(wk_d, 1, "wk")

                for tt in (2 * s, 2 * s + 1):
                    if tt != 0:
                        load_x(tt)
                    if tt == 2:
                        wv_t[1] = load_w(wv_d, 1, "wv", tag_s=False)
                    if tt == 0:
                        emit_v(0, 0)
                        emit_qk_pairs(0, 0)
                    else:
                        emit_qk(tt, s)
                        emit_v(tt, s)

        # ------- Phase C+D: attention + output projection, interleaved -----
        with ExitStack() as att:
            sps_ps = att.enter_context(tc.tile_pool(name="spsps", bufs=2, space="PSUM"))
            oacc_ps = att.enter_context(tc.tile_pool(name="oaccps", bufs=1, space="PSUM"))
            sums_ps = att.enter_context(tc.tile_pool(name="sumsps", bufs=1, space="PSUM"))
            out_ps = att.enter_context(tc.tile_pool(name="outps", bufs=2, space="PSUM"))
            es_pool = att.enter_context(tc.tile_pool(name="es", bufs=6))
            sacc_pool = att.enter_context(tc.tile_pool(name="sacc", bufs=2))
            sc2_pool = att.enter_context(tc.tile_pool(name="sc2", bufs=2))
            rc_pool = att.enter_context(tc.tile_pool(name="rc", bufs=2))
            on_pool = att.enter_context(tc.tile_pool(name="onorm", bufs=2))
            osb_pool = att.enter_context(tc.tile_pool(name="osb", bufs=2))
            wo_pool = att.enter_context(tc.tile_pool(name="wopool", bufs=1))

            # wo rides the SP queue: a dma_start costs ~660ns on the issuing
            # engine's sequencer, and the Act queue must reach the first exp
            # activations immediately at phase start.
            wo_t = [wo_pool.tile([128, HPC * D], BF, name=f"wos{s}", tag=f"wo{s}")
                    for s in range(2)]
            for s in range(2):
                for hd in range(HPC):
                    nc.sync.dma_start(wo_t[s][:, hd * D:(hd + 1) * D], wo_d[s, hd])

            onorm = {}

            def outproj_emitters(qt, tail=False):
                """One closure per output-projection MATMUL for query tile qt
                (4 per od-group); the psum copy + DMA ride on the 4th.  In the
                tail (last qt, nothing left to interleave with) rotate the od
                psums across the three same-sized psum pools so the Pool
                copies overlap the next groups' matmuls."""
                s = 0 if qt < 2 else 1
                qsl = slice(qt * 512, (qt + 1) * 512)
                pools = [(out_ps, "ops")]
                cell = {}
                items = []
                for od in range(NKC):
                    for hd in range(HPC):
                        def emit(od=od, hd=hd):
                            if hd == 0:
                                pool, tag = pools[od % len(pools)]
                                cell["ops"] = pool.tile(
                                    [128, 512], F32, name=f"op{qt}_{od}", tag=tag)
                            ops = cell["ops"]
                            nc.tensor.matmul(
                                ops[:],
                                wo_t[s][:, hd * D + od * 128: hd * D + (od + 1) * 128],
                                onorm[(hd, qt)][:],
                                start=(hd == 0), stop=(hd == HPC - 1))
                            if hd == HPC - 1:
                                if od % 4 == 0:
                                    cell["osb"] = osb_pool.tile(
                                        [128, 4, 512], BF, name=f"ou{qt}_{od}", tag="osb")
                                osb = cell["osb"]
                                nc.vector.tensor_copy(osb[:, od % 4, :], ops[:])
                                if od % 2 == 1:
                                    nc.sync.dma_start(
                                        out_d[:, qt, od - 1:od + 1, :],
                                        osb[:, (od % 4) - 1:(od % 4) + 1, :])
                        items.append(emit)
                return items

            pending = []
            for qt in range(NTT):
                qsl = slice(qt * 512, (qt + 1) * 512)
                for h in range(HPC):
                    hsl = slice(h * DH, (h + 1) * DH)
                    oacc = oacc_ps.tile([128, 512], F32, name=f"oa{h}{qt}", tag="oacc")
                    sacc = sacc_pool.tile([128, 2, 512], BF, name=f"sa{h}{qt}", tag="sacc")
                    for p in range(NKC // 2):
                        sps = sps_ps.tile([128, 2, 512], F32, name=f"sp{h}{qt}{p}", tag="sps")
                        for plane in range(2):
                            ksl = slice((2 * p + plane) * 128, (2 * p + plane + 1) * 128)
                            nc.tensor.matmul(sps[:, plane, :], qk_rope[h][:, 1, ksl],
                                             qk_rope[h][:, 0, qsl], start=True, stop=True)
                        es = es_pool.tile([128, 2, 512], BF, name=f"es{h}{qt}{p}", tag="es")
                        nc.scalar.activation(es[:], sps[:], AF.Exp, bias=zero_t[:, 0:1])
                        for plane in range(2):
                            nc.tensor.matmul(oacc[:], v_pair[p][:, plane, hsl],
                                             es[:, plane, :],
                                             start=(p == 0 and plane == 0),
                                             stop=(p == NKC // 2 - 1 and plane == 1))
                        if p == 0:
                            nc.vector.tensor_copy(sacc[:], es[:])
                        else:
                            nc.vector.tensor_tensor(sacc[:], sacc[:], es[:], ALU.add)
                        for _ in range(2):
                            if pending:
                                pending.pop(0)()
                    sc2 = sc2_pool.tile([128, 512], BF, name=f"sc{h}{qt}", tag="sc2")
                    nc.vector.tensor_tensor(sc2[:], sacc[:, 0, :], sacc[:, 1, :], ALU.add)
                    sums = sums_ps.tile([128, 512], F32, name=f"su{h}{qt}", tag="sums")
                    nc.tensor.matmul(sums[:], ones_t[:], sc2[:], start=True, stop=True)
                    rc = rc_pool.tile([128, 512], F32, name=f"rc{h}{qt}", tag="rc")
                    nc.vector.reciprocal(rc[:], sums[:])
                    on_t = on_pool.tile([128, 512], BF, name=f"on{h}{qt}", tag=f"on{h}")
                    nc.vector.tensor_tensor(on_t[:], oacc[:], rc[:], ALU.mult)
                    onorm[(h, qt)] = on_t
                while pending:
                    pending.pop(0)()
                pending = outproj_emitters(qt, tail=(qt == NTT - 1))
            while pending:
                pending.pop(0)()


def shard_inputs(inputs):
    """Full inputs -> per-core in_maps (host-side prep: transpose, cast,
    scale-folding, per-head slicing)."""
    f32 = np.float32
    x1, x2 = np.asarray(inputs["x_1"], f32), np.asarray(inputs["x_2"], f32)
    cosT = np.ascontiguousarray(
        np.concatenate([np.asarray(inputs["cos1"]), np.asarray(inputs["cos2"])], 0).T
    ).astype(bf16)
    # sign of the RoPE rotation folded in: row p gets -sin for even p
    # (out[2i] = q[2i]*cos - q[2i+1]*sin), +sin for odd p
    sign = np.where(np.arange(128) % 2 == 0, -1.0, 1.0).astype(f32)[:, None]
    sinT = np.ascontiguousarray(
        np.concatenate([np.asarray(inputs["sin1"]), np.asarray(inputs["sin2"])], 0).T
        * sign
    ).astype(bf16)

    in_maps = []
    for c in range(N_CORES):
        b, hg = divmod(c, 4)
        hsl = slice(hg * HPC * DH, (hg + 1) * HPC * DH)
        xc = np.concatenate([x1[b], x2[b]], 0)          # [T, D]
        # [128, NTT, NKC, 512]: xT[p, tt, kc, j] = xc[tt*512+j, kc*128+p]
        xT = np.ascontiguousarray(
            xc.reshape(NTT, 512, NKC, 128).transpose(3, 0, 2, 1)).astype(bf16)

        def wslice(name, scale=1.0):
            # [128, 2, NKC, 512]: w[p, s, kc, f] = w_s[kc*128+p, hsl.start+f]
            out = np.empty((128, 2, NKC, HPC * DH), bf16)
            for s in range(2):
                w = np.asarray(inputs[name + str(s + 1)], f32)[:, hsl] * scale
                out[:, s] = w.reshape(NKC, 128, HPC * DH).transpose(1, 0, 2).astype(bf16)
            return out

        wq = wslice("wq", SCALE)
        wk = wslice("wk")
        wv = wslice("wv")
        wo = np.empty((2, HPC, 128, D), bf16)
        for s in range(2):
            wo[s] = np.asarray(inputs["wo" + str(s + 1)], f32)[hsl, :].astype(bf16).reshape(HPC, 128, D)

        bias = np.zeros((128, 17), f32)
        bias[:, 16] = np.where(np.arange(128) % 2 == 0, -1.0, 1.0)
        for s in range(2):
            bqs = np.asarray(inputs["bq" + str(s + 1)], f32)[hsl] * SCALE
            bks = np.asarray(inputs["bk" + str(s + 1)], f32)[hsl]
            for h in range(HPC):
                bias[:, s * 8 + h] = bqs[h * DH:(h + 1) * DH]
                bias[:, s * 8 + 4 + h] = bks[h * DH:(h + 1) * DH]
        # bvb[p, s, plane, f] = bv_s[f]  (broadcast over partitions/planes)
        bvb = np.empty((128, 2, 2, 512), bf16)
        for s in range(2):
            bvs = np.asarray(inputs["bv" + str(s + 1)], f32)[hsl].astype(bf16)
            bvb[:, s] = np.broadcast_to(bvs, (128, 2, 512))

        in_maps.append({
            "xT": xT, "wq": wq, "wk": wk, "wv": wv, "wo": wo,
            "bias_qk": bias, "bvb": bvb, "cosT": cosT, "sinT": sinT,
        })
    return in_maps


def unshard_outputs(results, inputs):
    f32 = np.float32
    acc = np.zeros((B, D, T), f32)
    for c in range(N_CORES):
        # outT [128, NTT, NKC, 512] -> [D, T]: out[od*128+p, qt*512+j]
        r = results[c]["outT"].astype(f32)
        acc[c // 4] += r.transpose(2, 0, 1, 3).reshape(D, T)
    o1 = np.empty((B, N1, D), f32)
    o2 = np.empty((B, N2, D), f32)
    bo1 = np.asarray(inputs["bo1"], f32)
    bo2 = np.asarray(inputs["bo2"], f32)
    for b in range(B):
        full = acc[b].T                                  # [T, D]
        o1[b] = full[:N1] + bo1
        o2[b] = full[N1:] + bo2
    return o1, o2


def kernel(**inputs):
    nc = build_program()
    in_maps = shard_inputs(inputs)
    res = run_bass_kernel_spmd(nc, in_maps, list(range(N_CORES)))
    return unshard_outputs(res.results, inputs)


if __name__ == "__main__":
    data = np.load("/root/problem/cache_inputs.npz")
    out = kernel(**{k: data[k] for k in data.files})
    exp = np.load("/root/problem/cache_expected.npz")
    for i, o in enumerate(out):
        e = exp[f"o{i+1}"]
        d = np.abs(o - e).max()
        print(f"o{i+1}: absmax_err {d:.4e} rel {d / np.abs(e).max():.4e}")



# revision 46
# speedup vs baseline: 1.0599x; 1.0599x over previous
"""Dual-stream multi-head attention on 8 Trainium2 NeuronCores (Bass/Tile).

Sharding: core c handles batch b = c//4 and head-group g = c%4 (4 of 16 heads).
Each core computes QKV projections (per-stream weights), RoPE, joint attention
over both streams, and a partial output projection (its heads' rows of wo).
The host sums the 4 per-core partials of each batch (bf16 partials, fp32
accumulate), transposes, and adds the output bias (with the v-bias's constant
contribution bv @ wo folded in host-side; softmax weights sum to 1 per stream
segment, so bv is added on-device per key-chunk via one broadcast-tile DVE add
per v psum instead of per-plane ones-matmuls).

Layout: fully transposed on-chip (features on partitions, tokens on the free
dim).  Scores are computed as S^T = k_rope @ q_rope^T so the PV matmul
consumes exp(S^T) with v in natural [token, dh] layout.

This revision runs at ~90% PE occupancy with 1584 matmuls x 213ns — within
~10us of the bf16 streaming floor.  Key structure:
- ALL inputs are laid out partition-major on the host ([128, ...] with
  contiguous bytes per partition) so every DMA runs at full pipe rate; the
  "k p f -> p k f" on-the-fly rearrange only reaches ~180 GB/s.
- ALL input DMAs ride the SP queue in exact global consumption order (the
  DMA pipe is one shared ~350 GB/s resource; a dma_start also costs ~660ns
  on the issuing engine's sequencer, so the Act queue carries none and
  reaches the attention exps immediately at phase-C start; wo rides SP).
- tt0 runs its v-projection first (1 weight chunk per 4 matmuls matches the
  piece-DMA delivery rate), with 2-chunk x/wv pieces: PE starts ~2.5us in;
  a short dummy-matmul warmup before that keeps the HAM clock-gate hot.
- tt0's q/k runs chunk-major over head PAIRS (2 psum tiles) so its weight
  consumption (852ns/chunk) never outruns the wq/wk piece stream.
- q+k share a [128,2,512] psum/sbuf pair per head; RoPE rotation stays on
  the PE (2 matmuls/head/tile; SBUF partition-pair-swap DMAs measured ~30us
  SLOWER on real HW than the sim claims); the rotation psum is read
  directly by the DVE sin-multiply (no Act copy).
- softmax denominators: per-head bf16 plane-fold on DVE then ONE ones-
  matmul (was two).
- exp'd score psums in 2-bank pairs, one Act instruction per 1024 elements;
  output projection interleaved 2-matmuls-per-pair-step into the next
  query-tile's attention, with fillers saved for the last tile's tail.
"""

import sys
import numpy as np

sys.path.insert(0, "/opt/trn_rl_repo")

import ml_dtypes
import concourse.bass as bass
import concourse.mybir as mybir
import concourse.tile as tile
from concourse.bass_utils import run_bass_kernel_spmd
from contextlib import ExitStack

B, N1, N2, D, H = 2, 1024, 1024, 2048, 16
T = N1 + N2              # 2048 tokens (both streams, concatenated)
DH = D // H              # 128
HPC = 4                  # heads per core
NKC = D // 128           # 16 contraction chunks
NTT = T // 512           # 4 512-token tiles
SCALE = DH ** -0.5
N_CORES = 8

BF = mybir.dt.bfloat16
F32 = mybir.dt.float32
bf16 = ml_dtypes.bfloat16
AF = mybir.ActivationFunctionType
ALU = mybir.AluOpType

_BUILT = {}


def build_program(repeats=1, phases="ABCD"):
    global _BUILT
    key = (repeats, phases)
    if key in _BUILT:
        return _BUILT[key]

    nc = bass.Bass()

    # all big tensors are laid out partition-major on the host so every DMA
    # reads/writes contiguous bytes per partition (the "k p f -> p k f"
    # rearrange pattern only reaches ~180 GB/s; these hit full rate)
    xT_d = nc.dram_tensor("xT", [128, NTT, NKC, 512], BF, kind="ExternalInput")
    wq_d = nc.dram_tensor("wq", [128, 2, NKC, HPC * DH], BF, kind="ExternalInput")
    wk_d = nc.dram_tensor("wk", [128, 2, NKC, HPC * DH], BF, kind="ExternalInput")
    wv_d = nc.dram_tensor("wv", [128, 2, NKC, HPC * DH], BF, kind="ExternalInput")
    wo_d = nc.dram_tensor("wo", [2, HPC, 128, D], BF, kind="ExternalInput")
    bias_d = nc.dram_tensor("bias_qk", [128, 17], F32, kind="ExternalInput")
    bvb_d = nc.dram_tensor("bvb", [128, 2, 2, 512], BF, kind="ExternalInput")
    cos_d = nc.dram_tensor("cosT", [128, T], BF, kind="ExternalInput")
    sin_d = nc.dram_tensor("sinT", [128, T], BF, kind="ExternalInput")
    rt_d = nc.dram_tensor("Rt", [128, 128], BF, kind="ExternalInput")
    out_d = nc.dram_tensor("outT", [128, NTT, NKC, 512], BF, kind="ExternalOutput")

    with tile.TileContext(nc) as tc:
        for _ in range(repeats):
            _emit(tc, nc, xT_d, wq_d, wk_d, wv_d, wo_d, bias_d, bvb_d, cos_d,
                  sin_d, rt_d, out_d, phases=phases)

    _split_dma_waits(nc)
    _BUILT[key] = nc
    return nc


def _split_dma_waits(nc):
    """This walrus build's 64-byte instruction encoding holds exactly one sync
    wait; peel extras into standalone EventSemaphore waits on the same
    engine immediately before the instruction."""
    wid = 0
    fn = nc.m.functions[0]
    for blk in fn.blocks:
        insts = blk.instructions
        out = []
        changed = False
        for inst in insts:
            si = inst.sync_info
            if si is not None and len(si.on_wait) > 1:
                waits = list(si.on_wait)
                for w in waits[:-1]:
                    pre = mybir.InstEventSemaphore(
                        name=f"WSPLIT-{wid}", ins=[], outs=[])
                    wid += 1
                    pre.engine = inst.engine
                    pre.sync_info = mybir.SyncInfo(on_wait=[w], on_update=[])
                    nc.register_instruction(pre, overwrite=True)
                    out.append(pre)
                inst.sync_info = mybir.SyncInfo(
                    on_wait=[waits[-1]], on_update=list(si.on_update))
                changed = True
            out.append(inst)
        if changed:
            blk.instructions = out


def _emit(tc, nc, xT_d, wq_d, wk_d, wv_d, wo_d, bias_d, bvb_d, cos_d, sin_d,
          rt_d, out_d, phases="ABCD"):
    with ExitStack() as top:
        consts = top.enter_context(tc.tile_pool(name="consts", bufs=1))
        persist = top.enter_context(tc.tile_pool(name="persist", bufs=1))

        rt_t = consts.tile([128, 128], BF, name="rt_t", tag="rt_t")
        bias_t = consts.tile([128, 17], F32, name="bias_t", tag="bias_t")
        bvb_t = consts.tile([128, 2, 2, 512], BF, name="bvb_t", tag="bvb_t")
        ones_t = consts.tile([128, 128], BF, name="ones_t", tag="ones_t")
        nc.vector.memset(ones_t[:], 1.0)
        zero_t = consts.tile([128, 1], F32, name="zero_t", tag="zero_t")
        nc.vector.memset(zero_t[:], 0.0)
        cosT = consts.tile([128, T], BF, name="cosT_t", tag="cosT_t")
        sinT = consts.tile([128, T], BF, name="sinT_t", tag="sinT_t")

        # qk_rope[h]: plane 0 = q_rope, plane 1 = k_rope  (bf16, [128, 2, T])
        qk_rope = [persist.tile([128, 2, T], BF, name=f"qkr{h}", tag=f"qkr{h}")
                   for h in range(HPC)]
        # v pairs: v_pair[p] covers token chunks (2p, 2p+1), natural layout
        v_pair = [persist.tile([128, 2, HPC * DH], BF, name=f"vp{p}", tag=f"vp{p}")
                  for p in range(NKC // 2)]

        # ---------------- Phase A: q,k projections + RoPE ------------------
        # ---------------- Phase B: v (natural layout) ----------------------
        with ExitStack() as ab:
            # per-stream full weight tiles [128, 16, 512] (tags carry s so
            # stream-1 DMAs prefetch during stream 0 without WAR waits); wv
            # reuses one tag (its s=1 DMA WAR-waits v(tt1)).  Stream-0 wq/wk
            # and the tt0 x tile are delivered as 2-chunk pieces interleaved
            # in consumption order so PE starts ~1us in.
            w_pool = ab.enter_context(tc.tile_pool(name="w", bufs=1))
            x_pool = ab.enter_context(tc.tile_pool(name="xs", bufs=2))
            x0_pool = ab.enter_context(tc.tile_pool(name="x0", bufs=1))
            sb_pool = ab.enter_context(tc.tile_pool(name="sb", bufs=2))
            t_pool = ab.enter_context(tc.tile_pool(name="tt", bufs=2))
            # one psum pool, 4 two-bank tags: qkp0/qkp1 alternate across
            # heads; rv0/rv1 are shared by the rotation psums (qk block) and
            # the v psums (v block) whose lifetimes interleave
            a_ps = ab.enter_context(tc.tile_pool(name="aps", bufs=1, space="PSUM"))

            HC = NKC // 2    # chunks per half-slab

            xt = {}          # tt -> (lo, hi) tiles [128, 8, 512]; tt=0: pieces
            x0p = []         # tt0 as 8 [128, 2, 512] piece tiles

            def xsl(tt, kc, toff=0, tn=512):
                if tt == 0:
                    return x0p[kc // 2][:, kc % 2, toff:toff + tn]
                return xt[tt][kc // HC][:, kc % HC, toff:toff + tn]

            def load_x(tt):
                # tt != 0 only (tt0 pieces are interleaved in the s==0 block)
                halves = []
                for i, nmi in ((0, "lo"), (1, "hi")):
                    t = x_pool.tile([128, HC, 512], BF, name=f"x{tt}{nmi}", tag=f"x{nmi}")
                    nc.sync.dma_start(t[:], xT_d[:, tt, i * HC:(i + 1) * HC, :])
                    halves.append(t)
                xt[tt] = halves

            def load_w(dram, s, tag, tag_s=True):
                t = w_pool.tile([128, NKC, HPC * DH], BF, name=f"{tag}{s}",
                                tag=f"{tag}{s}" if tag_s else tag)
                for i in range(2):
                    nc.sync.dma_start(t[:, i * HC:(i + 1) * HC, :],
                                      dram[:, s, i * HC:(i + 1) * HC, :])
                return t

            def emit_v(tt, s):
                for pl in range(2):
                    vp = tt * 2 + pl
                    vps = a_ps.tile([128, 2, 512], F32, name=f"vps{vp}", tag=f"rv{pl}")
                    for plane in range(2):
                        toff = pl * 256 + plane * 128
                        for kc in range(NKC):
                            nc.tensor.matmul(
                                vps[:, plane, :], xsl(tt, kc, toff, 128),
                                wv_t[s][:, kc, :],
                                start=(kc == 0), stop=(kc == NKC - 1))
                    # + bv (broadcast tile), psum -> sbuf on DVE
                    nc.vector.tensor_tensor(v_pair[vp][:], vps[:], bvb_t[:, s],
                                            ALU.add)

            def rope_tail(tt, s, h, qkp_or_sb):
                tsl = slice(tt * 512, (tt + 1) * 512)
                qkp = qkp_or_sb
                # sb2: plane 0 = q+bias, plane 1 = k+bias (contiguous)
                sb2 = sb_pool.tile([128, 2, 512], BF, name=f"sb2{tt}{h}", tag="sb2")
                nc.scalar.activation(sb2[:, 0, :], qkp[:, 0, :], AF.Identity,
                                     bias=bias_t[:, s * 8 + h:s * 8 + h + 1])
                nc.scalar.activation(sb2[:, 1, :], qkp[:, 1, :], AF.Identity,
                                     bias=bias_t[:, s * 8 + 4 + h:s * 8 + 4 + h + 1])
                sw = a_ps.tile([128, 2, 512], F32, name=f"rp{tt}{h}", tag=f"rv{h % 2}")
                nc.tensor.matmul(sw[:, 0, :], rt_t[:], sb2[:, 0, :], start=True, stop=True)
                nc.tensor.matmul(sw[:, 1, :], rt_t[:], sb2[:, 1, :], start=True, stop=True)
                t12 = t_pool.tile([128, 2, 512], BF, name=f"t12_{tt}{h}", tag="t12")
                nc.vector.tensor_tensor(t12[:, 0, :], sb2[:, 0, :], cosT[:, tsl], ALU.mult)
                nc.vector.tensor_tensor(t12[:, 1, :], sb2[:, 1, :], cosT[:, tsl], ALU.mult)
                t3 = t_pool.tile([128, 2, 512], BF, name=f"t3_{tt}{h}", tag="t3")
                nc.vector.tensor_tensor(t3[:, 0, :], sw[:, 0, :], sinT[:, tsl], ALU.mult)
                nc.vector.tensor_tensor(t3[:, 1, :], sw[:, 1, :], sinT[:, tsl], ALU.mult)
                nc.vector.tensor_tensor(qk_rope[h][:, :, tsl], t12[:], t3[:], ALU.add)

            def emit_qk(tt, s):
                for h in range(HPC):
                    hsl = slice(h * DH, (h + 1) * DH)
                    qkp = a_ps.tile([128, 2, 512], F32, name=f"qkp{tt}{h}",
                                    tag=f"qkp{h % 2}")
                    for kc in range(NKC):
                        nc.tensor.matmul(qkp[:, 0, :], wq_t[s][:, kc, hsl],
                                         xsl(tt, kc),
                                         start=(kc == 0), stop=(kc == NKC - 1))
                        nc.tensor.matmul(qkp[:, 1, :], wk_t[s][:, kc, hsl],
                                         xsl(tt, kc),
                                         start=(kc == 0), stop=(kc == NKC - 1))
                    rope_tail(tt, s, h, qkp)

            def emit_qk_pairs(tt, s):
                # chunk-major over head pairs: consumes each weight chunk at
                # 852ns vs the ~730ns/chunk DMA pipe delivery, so tt0's q/k
                # never outruns the interleaved wq/wk piece stream.
                for hp in range(2):
                    pair = (2 * hp, 2 * hp + 1)
                    qkps = [a_ps.tile([128, 2, 512], F32, name=f"qkp{tt}{h}",
                                       tag=f"qkp{h % 2}") for h in pair]
                    for kc in range(NKC):
                        for i, h in enumerate(pair):
                            hsl = slice(h * DH, (h + 1) * DH)
                            nc.tensor.matmul(qkps[i][:, 0, :], wq_t[s][:, kc, hsl],
                                             xsl(tt, kc),
                                             start=(kc == 0), stop=(kc == NKC - 1))
                            nc.tensor.matmul(qkps[i][:, 1, :], wk_t[s][:, kc, hsl],
                                             xsl(tt, kc),
                                             start=(kc == 0), stop=(kc == NKC - 1))
                    for i, h in enumerate(pair):
                        rope_tail(tt, s, h, qkps[i])

            # PE warmup: dummy matmuls from ~0.4us until the first v-chunk
            # DMAs land (~2.6us) keep the HAM activity window hot so real
            # matmuls start at full clock.
            wps = a_ps.tile([128, 512], F32, name="warm", tag="qkp0")
            for _ in range(20):
                nc.tensor.matmul(wps[:, 0:128], ones_t[:], ones_t[:],
                                 start=True, stop=True)

            wq_t, wk_t, wv_t = {}, {}, {}
            for s in range(2):
                if s == 0:
                    # ALL input DMAs ride ONE queue (SP) in exact global
                    # consumption order — the DMA pipe is a single shared
                    # ~350 GB/s resource, so cross-queue interleaving only
                    # scrambles the arrival order.  tt0 runs its
                    # v-projection FIRST (consumes 1 chunk per 4 matmuls =
                    # 852ns vs 730ns/chunk delivery) while wq/wk stream in
                    # behind wv; the Act queue carries no DMAs at all (a
                    # dma_start costs ~660ns on the issuing sequencer).
                    wv_t[0] = w_pool.tile([128, NKC, HPC * DH], BF, name="wv0",
                                          tag="wv")
                    for pc in range(8):
                        csl = slice(2 * pc, 2 * pc + 2)
                        t = x0_pool.tile([128, 2, 512], BF, name=f"x0p{pc}",
                                         tag=f"x0p{pc}")
                        nc.sync.dma_start(t[:], xT_d[:, 0, csl, :])
                        x0p.append(t)
                        nc.sync.dma_start(wv_t[0][:, csl, :], wv_d[:, 0, csl, :])
                    nc.sync.dma_start(bvb_t[:], bvb_d[:])
                    nc.sync.dma_start(rt_t[:], rt_d[:])
                    nc.sync.dma_start(bias_t[:], bias_d[:])
                    # wq/wk interleaved 2-chunk pieces in tt0's kc
                    # consumption order
                    wq_t[0] = w_pool.tile([128, NKC, HPC * DH], BF, name="wq0",
                                          tag="wq0")
                    wk_t[0] = w_pool.tile([128, NKC, HPC * DH], BF, name="wk0",
                                          tag="wk0")
                    for pc in range(8):
                        csl = slice(2 * pc, 2 * pc + 2)
                        nc.sync.dma_start(wq_t[0][:, csl, :], wq_d[:, 0, csl, :])
                        nc.sync.dma_start(wk_t[0][:, csl, :], wk_d[:, 0, csl, :])
                    nc.sync.dma_start(cosT[:], cos_d[:])
                    nc.sync.dma_start(sinT[:], sin_d[:])
                else:
                    wq_t[1] = load_w(wq_d, 1, "wq")
                    wk_t[1] = load_w(wk_d, 1, "wk")

                for tt in (2 * s, 2 * s + 1):
                    if tt != 0:
                        load_x(tt)
                    if tt == 2:
                        wv_t[1] = load_w(wv_d, 1, "wv", tag_s=False)
                    if tt == 0:
                        emit_v(0, 0)
                        emit_qk_pairs(0, 0)
                    else:
                        emit_qk(tt, s)
                        emit_v(tt, s)

        # ------- Phase C+D: attention + output projection, interleaved -----
        with ExitStack() as att:
            sps_ps = att.enter_context(tc.tile_pool(name="spsps", bufs=2, space="PSUM"))
            oacc_ps = att.enter_context(tc.tile_pool(name="oaccps", bufs=1, space="PSUM"))
            sums_ps = att.enter_context(tc.tile_pool(name="sumsps", bufs=1, space="PSUM"))
            out_ps = att.enter_context(tc.tile_pool(name="outps", bufs=2, space="PSUM"))
            es_pool = att.enter_context(tc.tile_pool(name="es", bufs=6))
            sacc_pool = att.enter_context(tc.tile_pool(name="sacc", bufs=2))
            sc2_pool = att.enter_context(tc.tile_pool(name="sc2", bufs=2))
            rc_pool = att.enter_context(tc.tile_pool(name="rc", bufs=2))
            on_pool = att.enter_context(tc.tile_pool(name="onorm", bufs=2))
            osb_pool = att.enter_context(tc.tile_pool(name="osb", bufs=2))
            wo_pool = att.enter_context(tc.tile_pool(name="wopool", bufs=1))

            # wo rides the SP queue: a dma_start costs ~660ns on the issuing
            # engine's sequencer, and the Act queue must reach the first exp
            # activations immediately at phase start.
            wo_t = [wo_pool.tile([128, HPC * D], BF, name=f"wos{s}", tag=f"wo{s}")
                    for s in range(2)]
            for s in range(2):
                for hd in range(HPC):
                    nc.sync.dma_start(wo_t[s][:, hd * D:(hd + 1) * D], wo_d[s, hd])

            onorm = {}

            def outproj_emitters(qt, tail=False):
                """One closure per output-projection MATMUL for query tile qt
                (4 per od-group); the psum copy + DMA ride on the 4th.  In the
                tail (last qt, nothing left to interleave with) rotate the od
                psums across the three same-sized psum pools so the Pool
                copies overlap the next groups' matmuls."""
                s = 0 if qt < 2 else 1
                qsl = slice(qt * 512, (qt + 1) * 512)
                pools = [(out_ps, "ops")]
                cell = {}
                items = []
                for od in range(NKC):
                    for hd in range(HPC):
                        def emit(od=od, hd=hd):
                            if hd == 0:
                                pool, tag = pools[od % len(pools)]
                                cell["ops"] = pool.tile(
                                    [128, 512], F32, name=f"op{qt}_{od}", tag=tag)
                            ops = cell["ops"]
                            nc.tensor.matmul(
                                ops[:],
                                wo_t[s][:, hd * D + od * 128: hd * D + (od + 1) * 128],
                                onorm[(hd, qt)][:],
                                start=(hd == 0), stop=(hd == HPC - 1))
                            if hd == HPC - 1:
                                if od % 4 == 0:
                                    cell["osb"] = osb_pool.tile(
                                        [128, 4, 512], BF, name=f"ou{qt}_{od}", tag="osb")
                                osb = cell["osb"]
                                nc.vector.tensor_copy(osb[:, od % 4, :], ops[:])
                                if qt == NTT - 1:
                                    # finest grain for the drain tail: the
                                    # final DMA trails the final matmul by
                                    # only ~0.4us
                                    nc.sync.dma_start(
                                        out_d[:, qt, od:od + 1, :],
                                        osb[:, od % 4:od % 4 + 1, :])
                                elif od % 2 == 1:
                                    nc.sync.dma_start(
                                        out_d[:, qt, od - 1:od + 1, :],
                                        osb[:, (od % 4) - 1:(od % 4) + 1, :])
                        items.append(emit)
                return items

            pending = []
            for qt in range(NTT):
                qsl = slice(qt * 512, (qt + 1) * 512)
                for h in range(HPC):
                    hsl = slice(h * DH, (h + 1) * DH)
                    oacc = oacc_ps.tile([128, 512], F32, name=f"oa{h}{qt}", tag="oacc")
                    sacc = sacc_pool.tile([128, 2, 512], BF, name=f"sa{h}{qt}", tag="sacc")
                    for p in range(NKC // 2):
                        sps = sps_ps.tile([128, 2, 512], F32, name=f"sp{h}{qt}{p}", tag="sps")
                        for plane in range(2):
                            ksl = slice((2 * p + plane) * 128, (2 * p + plane + 1) * 128)
                            nc.tensor.matmul(sps[:, plane, :], qk_rope[h][:, 1, ksl],
                                             qk_rope[h][:, 0, qsl], start=True, stop=True)
                        es = es_pool.tile([128, 2, 512], BF, name=f"es{h}{qt}{p}", tag="es")
                        nc.scalar.activation(es[:], sps[:], AF.Exp, bias=zero_t[:, 0:1])
                        for plane in range(2):
                            nc.tensor.matmul(oacc[:], v_pair[p][:, plane, hsl],
                                             es[:, plane, :],
                                             start=(p == 0 and plane == 0),
                                             stop=(p == NKC // 2 - 1 and plane == 1))
                        if p == 0:
                            nc.vector.tensor_copy(sacc[:], es[:])
                        else:
                            nc.vector.tensor_tensor(sacc[:], sacc[:], es[:], ALU.add)
                        # last qt: save fillers for its tail (the drain has
                        # no attention left to hide the sums/rc chains)
                        npop = 1 if (qt == NTT - 1 and h >= 2) else 2
                        for _ in range(npop):
                            if pending:
                                pending.pop(0)()
                    sc2 = sc2_pool.tile([128, 512], BF, name=f"sc{h}{qt}", tag="sc2")
                    nc.vector.tensor_tensor(sc2[:], sacc[:, 0, :], sacc[:, 1, :], ALU.add)
                    sums = sums_ps.tile([128, 512], F32, name=f"su{h}{qt}", tag="sums")
                    nc.tensor.matmul(sums[:], ones_t[:], sc2[:], start=True, stop=True)
                    rc = rc_pool.tile([128, 512], F32, name=f"rc{h}{qt}", tag="rc")
                    nc.vector.reciprocal(rc[:], sums[:])
                    on_t = on_pool.tile([128, 512], BF, name=f"on{h}{qt}", tag=f"on{h}")
                    nc.vector.tensor_tensor(on_t[:], oacc[:], rc[:], ALU.mult)
                    onorm[(h, qt)] = on_t
                while pending:
                    pending.pop(0)()
                pending = outproj_emitters(qt, tail=(qt == NTT - 1))
            while pending:
                pending.pop(0)()


def shard_inputs(inputs):
    """Full inputs -> per-core in_maps (host-side prep: transpose, cast,
    scale-folding, per-head slicing)."""
    f32 = np.float32
    x1, x2 = np.asarray(inputs["x_1"], f32), np.asarray(inputs["x_2"], f32)
    cosT = np.ascontiguousarray(
        np.concatenate([np.asarray(inputs["cos1"]), np.asarray(inputs["cos2"])], 0).T
    ).astype(bf16)
    sinT = np.ascontiguousarray(
        np.concatenate([np.asarray(inputs["sin1"]), np.asarray(inputs["sin2"])], 0).T
    ).astype(bf16)
    rt = np.zeros((128, 128), np.float32)
    idx = np.arange(0, 128, 2)
    rt[idx, idx + 1] = 1.0
    rt[idx + 1, idx] = -1.0
    rt = rt.astype(bf16)

    in_maps = []
    for c in range(N_CORES):
        b, hg = divmod(c, 4)
        hsl = slice(hg * HPC * DH, (hg + 1) * HPC * DH)
        xc = np.concatenate([x1[b], x2[b]], 0)          # [T, D]
        # [128, NTT, NKC, 512]: xT[p, tt, kc, j] = xc[tt*512+j, kc*128+p]
        xT = np.ascontiguousarray(
            xc.reshape(NTT, 512, NKC, 128).transpose(3, 0, 2, 1)).astype(bf16)

        def wslice(name, scale=1.0):
            # [128, 2, NKC, 512]: w[p, s, kc, f] = w_s[kc*128+p, hsl.start+f]
            out = np.empty((128, 2, NKC, HPC * DH), bf16)
            for s in range(2):
                w = np.asarray(inputs[name + str(s + 1)], f32)[:, hsl] * scale
                out[:, s] = w.reshape(NKC, 128, HPC * DH).transpose(1, 0, 2).astype(bf16)
            return out

        wq = wslice("wq", SCALE)
        wk = wslice("wk")
        wv = wslice("wv")
        wo = np.empty((2, HPC, 128, D), bf16)
        for s in range(2):
            wo[s] = np.asarray(inputs["wo" + str(s + 1)], f32)[hsl, :].astype(bf16).reshape(HPC, 128, D)

        bias = np.zeros((128, 17), f32)
        bias[:, 16] = np.where(np.arange(128) % 2 == 0, -1.0, 1.0)
        for s in range(2):
            bqs = np.asarray(inputs["bq" + str(s + 1)], f32)[hsl] * SCALE
            bks = np.asarray(inputs["bk" + str(s + 1)], f32)[hsl]
            for h in range(HPC):
                bias[:, s * 8 + h] = bqs[h * DH:(h + 1) * DH]
                bias[:, s * 8 + 4 + h] = bks[h * DH:(h + 1) * DH]
        # bvb[p, s, plane, f] = bv_s[f]  (broadcast over partitions/planes)
        bvb = np.empty((128, 2, 2, 512), bf16)
        for s in range(2):
            bvs = np.asarray(inputs["bv" + str(s + 1)], f32)[hsl].astype(bf16)
            bvb[:, s] = np.broadcast_to(bvs, (128, 2, 512))

        in_maps.append({
            "xT": xT, "wq": wq, "wk": wk, "wv": wv, "wo": wo,
            "bias_qk": bias, "bvb": bvb, "cosT": cosT, "sinT": sinT, "Rt": rt,
        })
    return in_maps


def unshard_outputs(results, inputs):
    f32 = np.float32
    acc = np.zeros((B, D, T), f32)
    for c in range(N_CORES):
        # outT [128, NTT, NKC, 512] -> [D, T]: out[od*128+p, qt*512+j]
        r = results[c]["outT"].astype(f32)
        acc[c // 4] += r.transpose(2, 0, 1, 3).reshape(D, T)
    o1 = np.empty((B, N1, D), f32)
    o2 = np.empty((B, N2, D), f32)
    bo1 = np.asarray(inputs["bo1"], f32)
    bo2 = np.asarray(inputs["bo2"], f32)
    for b in range(B):
        full = acc[b].T                                  # [T, D]
        o1[b] = full[:N1] + bo1
        o2[b] = full[N1:] + bo2
    return o1, o2


def kernel(**inputs):
    nc = build_program()
    in_maps = shard_inputs(inputs)
    res = run_bass_kernel_spmd(nc, in_maps, list(range(N_CORES)))
    return unshard_outputs(res.results, inputs)


if __name__ == "__main__":
    data = np.load("/root/problem/cache_inputs.npz")
    out = kernel(**{k: data[k] for k in data.files})
    exp = np.load("/root/problem/cache_expected.npz")
    for i, o in enumerate(out):
        e = exp[f"o{i+1}"]
        d = np.abs(o - e).max()
        print(f"o{i+1}: absmax_err {d:.4e} rel {d / np.abs(e).max():.4e}")



# revision 47
# speedup vs baseline: 1.1677x; 1.1017x over previous
"""Dual-stream multi-head attention on 8 Trainium2 NeuronCores (Bass/Tile).

Sharding: core c handles batch b = c//4 and head-group g = c%4 (4 of 16 heads).
Each core computes QKV projections (per-stream weights), RoPE, joint attention
over both streams, and a partial output projection (its heads' rows of wo).
The host sums the 4 per-core partials of each batch (bf16 partials, fp32
accumulate), transposes, and adds the output bias.

Layout: fully transposed on-chip (features on partitions, tokens on the free
dim).  Scores are computed as S^T = k_rope @ q_rope^T so the PV matmul
consumes exp(S^T) with v in natural [token, dh] layout.

This revision runs at ~90% PE occupancy with 1584 matmuls x 213ns -- within
~10us of the bf16 streaming floor.  Key structure:
- ALL inputs are laid out partition-major on the host ([128, ...] with
  contiguous bytes per partition) so every DMA runs at full pipe rate; the
  "k p f -> p k f" on-the-fly rearrange only reaches ~180 GB/s.
- ALL input DMAs ride the SP queue in exact global consumption order (the
  DMA pipe is one shared ~350 GB/s resource; a dma_start also costs ~660ns
  on the issuing engine's sequencer, so the Act queue carries none and
  reaches the attention exps immediately at phase-C start; wo rides SP).
- tt0 runs its v-projection first (1 weight chunk per 4 matmuls matches the
  piece-DMA delivery rate), with 2-chunk x/wv pieces: PE starts ~2.5us in;
  a short dummy-matmul warmup before that keeps the HAM clock-gate hot.
- tt0's q/k runs chunk-major over head PAIRS (2 psum tiles) so its weight
  consumption (852ns/chunk) never outruns the wq/wk piece stream.
- v-bias applied via one broadcast-tile DVE add per v psum (not per-plane
  ones-matmuls); softmax denominators fold the two bf16 sacc planes on the
  DVE then use ONE ones-matmul per head (was two).
- q+k share a [128,2,512] psum pair per head; RoPE rotation stays on the PE
  (2 matmuls/head/tile; SBUF partition-pair-swap DMAs measured ~30us SLOWER
  on real HW than the sim claims); the rotation psum is read directly by
  the DVE sin-multiply (no Act copy).
- exp'd score psums in 2-bank pairs, one Act instruction per 1024 elements;
  output projection interleaved 2-matmuls-per-pair-step into the next
  query-tile's attention; output as bf16 pairs, one DMA per 2 output-dim
  chunks on the SP queue.
"""

import sys
import numpy as np

sys.path.insert(0, "/opt/trn_rl_repo")

import ml_dtypes
import concourse.bass as bass
import concourse.mybir as mybir
import concourse.tile as tile
from concourse.bass_utils import run_bass_kernel_spmd
from contextlib import ExitStack

B, N1, N2, D, H = 2, 1024, 1024, 2048, 16
T = N1 + N2              # 2048 tokens (both streams, concatenated)
DH = D // H              # 128
HPC = 4                  # heads per core
NKC = D // 128           # 16 contraction chunks
NTT = T // 512           # 4 512-token tiles
SCALE = DH ** -0.5
N_CORES = 8

BF = mybir.dt.bfloat16
F32 = mybir.dt.float32
bf16 = ml_dtypes.bfloat16
AF = mybir.ActivationFunctionType
ALU = mybir.AluOpType

_BUILT = {}


def build_program(repeats=1, phases="ABCD"):
    global _BUILT
    key = (repeats, phases)
    if key in _BUILT:
        return _BUILT[key]

    nc = bass.Bass()

    # all big tensors are laid out partition-major on the host so every DMA
    # reads/writes contiguous bytes per partition (the "k p f -> p k f"
    # rearrange pattern only reaches ~180 GB/s; these hit full rate)
    xT_d = nc.dram_tensor("xT", [128, NTT, NKC, 512], BF, kind="ExternalInput")
    wq_d = nc.dram_tensor("wq", [128, 2, NKC, HPC * DH], BF, kind="ExternalInput")
    wk_d = nc.dram_tensor("wk", [128, 2, NKC, HPC * DH], BF, kind="ExternalInput")
    wv_d = nc.dram_tensor("wv", [128, 2, NKC, HPC * DH], BF, kind="ExternalInput")
    wo_d = nc.dram_tensor("wo", [2, HPC, 128, D], BF, kind="ExternalInput")
    bias_d = nc.dram_tensor("bias_qk", [128, 17], F32, kind="ExternalInput")
    bvb_d = nc.dram_tensor("bvb", [128, 2, 2, 512], BF, kind="ExternalInput")
    cos_d = nc.dram_tensor("cosT", [128, T], BF, kind="ExternalInput")
    sin_d = nc.dram_tensor("sinT", [128, T], BF, kind="ExternalInput")
    rt_d = nc.dram_tensor("Rt", [128, 128], BF, kind="ExternalInput")
    out_d = nc.dram_tensor("outT", [128, NTT, NKC, 512], BF, kind="ExternalOutput")

    with tile.TileContext(nc) as tc:
        for _ in range(repeats):
            _emit(tc, nc, xT_d, wq_d, wk_d, wv_d, wo_d, bias_d, bvb_d, cos_d,
                  sin_d, rt_d, out_d, phases=phases)

    _split_dma_waits(nc)
    _BUILT[key] = nc
    return nc


def _split_dma_waits(nc):
    """This walrus build's 64-byte instruction encoding holds exactly one sync
    wait; peel extras into standalone EventSemaphore waits on the same
    engine immediately before the instruction."""
    wid = 0
    fn = nc.m.functions[0]
    for blk in fn.blocks:
        insts = blk.instructions
        out = []
        changed = False
        for inst in insts:
            si = inst.sync_info
            if si is not None and len(si.on_wait) > 1:
                waits = list(si.on_wait)
                for w in waits[:-1]:
                    pre = mybir.InstEventSemaphore(
                        name=f"WSPLIT-{wid}", ins=[], outs=[])
                    wid += 1
                    pre.engine = inst.engine
                    pre.sync_info = mybir.SyncInfo(on_wait=[w], on_update=[])
                    nc.register_instruction(pre, overwrite=True)
                    out.append(pre)
                inst.sync_info = mybir.SyncInfo(
                    on_wait=[waits[-1]], on_update=list(si.on_update))
                changed = True
            out.append(inst)
        if changed:
            blk.instructions = out


def _emit(tc, nc, xT_d, wq_d, wk_d, wv_d, wo_d, bias_d, bvb_d, cos_d, sin_d,
          rt_d, out_d, phases="ABCD"):
    with ExitStack() as top:
        consts = top.enter_context(tc.tile_pool(name="consts", bufs=1))
        persist = top.enter_context(tc.tile_pool(name="persist", bufs=1))

        rt_t = consts.tile([128, 128], BF, name="rt_t", tag="rt_t")
        bias_t = consts.tile([128, 17], F32, name="bias_t", tag="bias_t")
        bvb_t = consts.tile([128, 2, 2, 512], BF, name="bvb_t", tag="bvb_t")
        ones_t = consts.tile([128, 128], BF, name="ones_t", tag="ones_t")
        nc.vector.memset(ones_t[:], 1.0)
        zero_t = consts.tile([128, 1], F32, name="zero_t", tag="zero_t")
        nc.vector.memset(zero_t[:], 0.0)
        cosT = consts.tile([128, T], BF, name="cosT_t", tag="cosT_t")
        sinT = consts.tile([128, T], BF, name="sinT_t", tag="sinT_t")

        # qk_rope[h]: plane 0 = q_rope, plane 1 = k_rope  (bf16, [128, 2, T])
        qk_rope = [persist.tile([128, 2, T], BF, name=f"qkr{h}", tag=f"qkr{h}")
                   for h in range(HPC)]
        # v pairs: v_pair[p] covers token chunks (2p, 2p+1), natural layout
        v_pair = [persist.tile([128, 2, HPC * DH], BF, name=f"vp{p}", tag=f"vp{p}")
                  for p in range(NKC // 2)]

        # ---------------- Phase A: q,k projections + RoPE ------------------
        # ---------------- Phase B: v (natural layout) ----------------------
        with ExitStack() as ab:
            # per-stream full weight tiles [128, 16, 512] (tags carry s so
            # stream-1 DMAs prefetch during stream 0 without WAR waits); wv
            # reuses one tag (its s=1 DMA WAR-waits v(tt1)).  Stream-0 wq/wk
            # and the tt0 x tile are delivered as 2-chunk pieces interleaved
            # in consumption order so PE starts ~1us in.
            w_pool = ab.enter_context(tc.tile_pool(name="w", bufs=1))
            x_pool = ab.enter_context(tc.tile_pool(name="xs", bufs=2))
            x0_pool = ab.enter_context(tc.tile_pool(name="x0", bufs=1))
            sb_pool = ab.enter_context(tc.tile_pool(name="sb", bufs=2))
            t_pool = ab.enter_context(tc.tile_pool(name="tt", bufs=2))
            # one psum pool, 4 two-bank tags: qkp0/qkp1 alternate across
            # heads; rv0/rv1 are shared by the rotation psums (qk block) and
            # the v psums (v block) whose lifetimes interleave
            a_ps = ab.enter_context(tc.tile_pool(name="aps", bufs=1, space="PSUM"))

            HC = NKC // 2    # chunks per half-slab

            xt = {}          # tt -> (lo, hi) tiles [128, 8, 512]; tt=0: pieces
            x0p = []         # tt0 as 8 [128, 2, 512] piece tiles

            def xsl(tt, kc, toff=0, tn=512):
                if tt == 0:
                    return x0p[kc // 2][:, kc % 2, toff:toff + tn]
                return xt[tt][kc // HC][:, kc % HC, toff:toff + tn]

            def load_x(tt):
                # tt != 0 only (tt0 pieces are interleaved in the s==0 block)
                halves = []
                for i, nmi in ((0, "lo"), (1, "hi")):
                    t = x_pool.tile([128, HC, 512], BF, name=f"x{tt}{nmi}", tag=f"x{nmi}")
                    nc.sync.dma_start(t[:], xT_d[:, tt, i * HC:(i + 1) * HC, :])
                    halves.append(t)
                xt[tt] = halves

            def load_w(dram, s, tag, tag_s=True):
                t = w_pool.tile([128, NKC, HPC * DH], BF, name=f"{tag}{s}",
                                tag=f"{tag}{s}" if tag_s else tag)
                for i in range(2):
                    nc.sync.dma_start(t[:, i * HC:(i + 1) * HC, :],
                                      dram[:, s, i * HC:(i + 1) * HC, :])
                return t

            def emit_v(tt, s):
                for pl in range(2):
                    vp = tt * 2 + pl
                    vps = a_ps.tile([128, 2, 512], F32, name=f"vps{vp}", tag=f"rv{pl}")
                    for plane in range(2):
                        toff = pl * 256 + plane * 128
                        for kc in range(NKC):
                            nc.tensor.matmul(
                                vps[:, plane, :], xsl(tt, kc, toff, 128),
                                wv_t[s][:, kc, :],
                                start=(kc == 0), stop=(kc == NKC - 1))
                    # + bv (broadcast tile), psum -> sbuf on DVE
                    nc.vector.tensor_tensor(v_pair[vp][:], vps[:], bvb_t[:, s],
                                            ALU.add)

            def rope_tail(tt, s, h, qkp_or_sb):
                tsl = slice(tt * 512, (tt + 1) * 512)
                qkp = qkp_or_sb
                # sb2: plane 0 = q+bias, plane 1 = k+bias (contiguous)
                sb2 = sb_pool.tile([128, 2, 512], BF, name=f"sb2{tt}{h}", tag="sb2")
                nc.scalar.activation(sb2[:, 0, :], qkp[:, 0, :], AF.Identity,
                                     bias=bias_t[:, s * 8 + h:s * 8 + h + 1])
                nc.scalar.activation(sb2[:, 1, :], qkp[:, 1, :], AF.Identity,
                                     bias=bias_t[:, s * 8 + 4 + h:s * 8 + 4 + h + 1])
                sw = a_ps.tile([128, 2, 512], F32, name=f"rp{tt}{h}", tag=f"rv{h % 2}")
                nc.tensor.matmul(sw[:, 0, :], rt_t[:], sb2[:, 0, :], start=True, stop=True)
                nc.tensor.matmul(sw[:, 1, :], rt_t[:], sb2[:, 1, :], start=True, stop=True)
                t12 = t_pool.tile([128, 2, 512], BF, name=f"t12_{tt}{h}", tag="t12")
                nc.vector.tensor_tensor(t12[:, 0, :], sb2[:, 0, :], cosT[:, tsl], ALU.mult)
                nc.vector.tensor_tensor(t12[:, 1, :], sb2[:, 1, :], cosT[:, tsl], ALU.mult)
                t3 = t_pool.tile([128, 2, 512], BF, name=f"t3_{tt}{h}", tag="t3")
                nc.vector.tensor_tensor(t3[:, 0, :], sw[:, 0, :], sinT[:, tsl], ALU.mult)
                nc.vector.tensor_tensor(t3[:, 1, :], sw[:, 1, :], sinT[:, tsl], ALU.mult)
                nc.vector.tensor_tensor(qk_rope[h][:, :, tsl], t12[:], t3[:], ALU.add)

            def emit_qk(tt, s):
                for h in range(HPC):
                    hsl = slice(h * DH, (h + 1) * DH)
                    qkp = a_ps.tile([128, 2, 512], F32, name=f"qkp{tt}{h}",
                                    tag=f"qkp{h % 2}")
                    for kc in range(NKC):
                        nc.tensor.matmul(qkp[:, 0, :], wq_t[s][:, kc, hsl],
                                         xsl(tt, kc),
                                         start=(kc == 0), stop=(kc == NKC - 1))
                        nc.tensor.matmul(qkp[:, 1, :], wk_t[s][:, kc, hsl],
                                         xsl(tt, kc),
                                         start=(kc == 0), stop=(kc == NKC - 1))
                    rope_tail(tt, s, h, qkp)

            def emit_qk_pairs(tt, s):
                # chunk-major over head pairs: consumes each weight chunk at
                # 852ns vs the ~730ns/chunk DMA pipe delivery, so tt0's q/k
                # never outruns the interleaved wq/wk piece stream.
                for hp in range(2):
                    pair = (2 * hp, 2 * hp + 1)
                    qkps = [a_ps.tile([128, 2, 512], F32, name=f"qkp{tt}{h}",
                                       tag=f"qkp{h % 2}") for h in pair]
                    for kc in range(NKC):
                        for i, h in enumerate(pair):
                            hsl = slice(h * DH, (h + 1) * DH)
                            nc.tensor.matmul(qkps[i][:, 0, :], wq_t[s][:, kc, hsl],
                                             xsl(tt, kc),
                                             start=(kc == 0), stop=(kc == NKC - 1))
                            nc.tensor.matmul(qkps[i][:, 1, :], wk_t[s][:, kc, hsl],
                                             xsl(tt, kc),
                                             start=(kc == 0), stop=(kc == NKC - 1))
                    for i, h in enumerate(pair):
                        rope_tail(tt, s, h, qkps[i])

            # PE warmup: dummy matmuls from ~0.4us until the first v-chunk
            # DMAs land (~2.6us) keep the HAM activity window hot so real
            # matmuls start at full clock.
            wps = a_ps.tile([128, 512], F32, name="warm", tag="qkp0")
            for _ in range(20):
                nc.tensor.matmul(wps[:, 0:128], ones_t[:], ones_t[:],
                                 start=True, stop=True)

            wq_t, wk_t, wv_t = {}, {}, {}
            for s in range(2):
                if s == 0:
                    # ALL input DMAs ride ONE queue (SP) in exact global
                    # consumption order — the DMA pipe is a single shared
                    # ~350 GB/s resource, so cross-queue interleaving only
                    # scrambles the arrival order.  tt0 runs its
                    # v-projection FIRST (consumes 1 chunk per 4 matmuls =
                    # 852ns vs 730ns/chunk delivery) while wq/wk stream in
                    # behind wv; the Act queue carries no DMAs at all (a
                    # dma_start costs ~660ns on the issuing sequencer).
                    wv_t[0] = w_pool.tile([128, NKC, HPC * DH], BF, name="wv0",
                                          tag="wv")
                    for pc in range(8):
                        csl = slice(2 * pc, 2 * pc + 2)
                        t = x0_pool.tile([128, 2, 512], BF, name=f"x0p{pc}",
                                         tag=f"x0p{pc}")
                        nc.sync.dma_start(t[:], xT_d[:, 0, csl, :])
                        x0p.append(t)
                        nc.sync.dma_start(wv_t[0][:, csl, :], wv_d[:, 0, csl, :])
                    nc.sync.dma_start(bvb_t[:], bvb_d[:])
                    nc.sync.dma_start(rt_t[:], rt_d[:])
                    nc.sync.dma_start(bias_t[:], bias_d[:])
                    # wq/wk interleaved 2-chunk pieces in tt0's kc
                    # consumption order
                    wq_t[0] = w_pool.tile([128, NKC, HPC * DH], BF, name="wq0",
                                          tag="wq0")
                    wk_t[0] = w_pool.tile([128, NKC, HPC * DH], BF, name="wk0",
                                          tag="wk0")
                    for pc in range(8):
                        csl = slice(2 * pc, 2 * pc + 2)
                        nc.sync.dma_start(wq_t[0][:, csl, :], wq_d[:, 0, csl, :])
                        nc.sync.dma_start(wk_t[0][:, csl, :], wk_d[:, 0, csl, :])
                    nc.sync.dma_start(cosT[:], cos_d[:])
                    nc.sync.dma_start(sinT[:], sin_d[:])
                else:
                    wq_t[1] = load_w(wq_d, 1, "wq")
                    wk_t[1] = load_w(wk_d, 1, "wk")

                for tt in (2 * s, 2 * s + 1):
                    if tt != 0:
                        load_x(tt)
                    if tt == 2:
                        wv_t[1] = load_w(wv_d, 1, "wv", tag_s=False)
                    if tt == 0:
                        emit_v(0, 0)
                        emit_qk_pairs(0, 0)
                    else:
                        emit_qk(tt, s)
                        emit_v(tt, s)

        # ------- Phase C+D: attention + output projection, interleaved -----
        with ExitStack() as att:
            sps_ps = att.enter_context(tc.tile_pool(name="spsps", bufs=2, space="PSUM"))
            oacc_ps = att.enter_context(tc.tile_pool(name="oaccps", bufs=1, space="PSUM"))
            sums_ps = att.enter_context(tc.tile_pool(name="sumsps", bufs=1, space="PSUM"))
            out_ps = att.enter_context(tc.tile_pool(name="outps", bufs=2, space="PSUM"))
            es_pool = att.enter_context(tc.tile_pool(name="es", bufs=6))
            sacc_pool = att.enter_context(tc.tile_pool(name="sacc", bufs=2))
            sc2_pool = att.enter_context(tc.tile_pool(name="sc2", bufs=2))
            rc_pool = att.enter_context(tc.tile_pool(name="rc", bufs=2))
            on_pool = att.enter_context(tc.tile_pool(name="onorm", bufs=2))
            osb_pool = att.enter_context(tc.tile_pool(name="osb", bufs=2))
            wo_pool = att.enter_context(tc.tile_pool(name="wopool", bufs=1))

            # wo rides the SP queue: a dma_start costs ~660ns on the issuing
            # engine's sequencer, and the Act queue must reach the first exp
            # activations immediately at phase start.
            wo_t = [wo_pool.tile([128, HPC * D], BF, name=f"wos{s}", tag=f"wo{s}")
                    for s in range(2)]
            for s in range(2):
                for hd in range(HPC):
                    nc.sync.dma_start(wo_t[s][:, hd * D:(hd + 1) * D], wo_d[s, hd])

            onorm = {}

            def outproj_emitters(qt, tail=False):
                """One closure per output-projection MATMUL for query tile qt
                (4 per od-group); the psum copy + DMA ride on the 4th.  In the
                tail (last qt, nothing left to interleave with) rotate the od
                psums across the three same-sized psum pools so the Pool
                copies overlap the next groups' matmuls."""
                s = 0 if qt < 2 else 1
                qsl = slice(qt * 512, (qt + 1) * 512)
                pools = [(out_ps, "ops")]
                cell = {}
                items = []
                for od in range(NKC):
                    for hd in range(HPC):
                        def emit(od=od, hd=hd):
                            if hd == 0:
                                pool, tag = pools[od % len(pools)]
                                cell["ops"] = pool.tile(
                                    [128, 512], F32, name=f"op{qt}_{od}", tag=tag)
                            ops = cell["ops"]
                            nc.tensor.matmul(
                                ops[:],
                                wo_t[s][:, hd * D + od * 128: hd * D + (od + 1) * 128],
                                onorm[(hd, qt)][:],
                                start=(hd == 0), stop=(hd == HPC - 1))
                            if hd == HPC - 1:
                                if od % 4 == 0:
                                    cell["osb"] = osb_pool.tile(
                                        [128, 4, 512], BF, name=f"ou{qt}_{od}", tag="osb")
                                osb = cell["osb"]
                                nc.vector.tensor_copy(osb[:, od % 4, :], ops[:])
                                if od % 2 == 1:
                                    nc.sync.dma_start(
                                        out_d[:, qt, od - 1:od + 1, :],
                                        osb[:, (od % 4) - 1:(od % 4) + 1, :])
                        items.append(emit)
                return items

            pending = []
            for qt in range(NTT):
                qsl = slice(qt * 512, (qt + 1) * 512)
                for h in range(HPC):
                    hsl = slice(h * DH, (h + 1) * DH)
                    oacc = oacc_ps.tile([128, 512], F32, name=f"oa{h}{qt}", tag="oacc")
                    sacc = sacc_pool.tile([128, 2, 512], BF, name=f"sa{h}{qt}", tag="sacc")
                    for p in range(NKC // 2):
                        sps = sps_ps.tile([128, 2, 512], F32, name=f"sp{h}{qt}{p}", tag="sps")
                        for plane in range(2):
                            ksl = slice((2 * p + plane) * 128, (2 * p + plane + 1) * 128)
                            nc.tensor.matmul(sps[:, plane, :], qk_rope[h][:, 1, ksl],
                                             qk_rope[h][:, 0, qsl], start=True, stop=True)
                        es = es_pool.tile([128, 2, 512], BF, name=f"es{h}{qt}{p}", tag="es")
                        nc.scalar.activation(es[:], sps[:], AF.Exp, bias=zero_t[:, 0:1])
                        for plane in range(2):
                            nc.tensor.matmul(oacc[:], v_pair[p][:, plane, hsl],
                                             es[:, plane, :],
                                             start=(p == 0 and plane == 0),
                                             stop=(p == NKC // 2 - 1 and plane == 1))
                        if p == 0:
                            nc.vector.tensor_copy(sacc[:], es[:])
                        else:
                            nc.vector.tensor_tensor(sacc[:], sacc[:], es[:], ALU.add)
                        for _ in range(2):
                            if pending:
                                pending.pop(0)()
                    sc2 = sc2_pool.tile([128, 512], BF, name=f"sc{h}{qt}", tag="sc2")
                    nc.vector.tensor_tensor(sc2[:], sacc[:, 0, :], sacc[:, 1, :], ALU.add)
                    sums = sums_ps.tile([128, 512], F32, name=f"su{h}{qt}", tag="sums")
                    nc.tensor.matmul(sums[:], ones_t[:], sc2[:], start=True, stop=True)
                    rc = rc_pool.tile([128, 512], F32, name=f"rc{h}{qt}", tag="rc")
                    nc.vector.reciprocal(rc[:], sums[:])
                    on_t = on_pool.tile([128, 512], BF, name=f"on{h}{qt}", tag=f"on{h}")
                    nc.vector.tensor_tensor(on_t[:], oacc[:], rc[:], ALU.mult)
                    onorm[(h, qt)] = on_t
                while pending:
                    pending.pop(0)()
                pending = outproj_emitters(qt, tail=(qt == NTT - 1))
            while pending:
                pending.pop(0)()


def shard_inputs(inputs):
    """Full inputs -> per-core in_maps (host-side prep: transpose, cast,
    scale-folding, per-head slicing)."""
    f32 = np.float32
    x1, x2 = np.asarray(inputs["x_1"], f32), np.asarray(inputs["x_2"], f32)
    cosT = np.ascontiguousarray(
        np.concatenate([np.asarray(inputs["cos1"]), np.asarray(inputs["cos2"])], 0).T
    ).astype(bf16)
    sinT = np.ascontiguousarray(
        np.concatenate([np.asarray(inputs["sin1"]), np.asarray(inputs["sin2"])], 0).T
    ).astype(bf16)
    rt = np.zeros((128, 128), np.float32)
    idx = np.arange(0, 128, 2)
    rt[idx, idx + 1] = 1.0
    rt[idx + 1, idx] = -1.0
    rt = rt.astype(bf16)

    in_maps = []
    for c in range(N_CORES):
        b, hg = divmod(c, 4)
        hsl = slice(hg * HPC * DH, (hg + 1) * HPC * DH)
        xc = np.concatenate([x1[b], x2[b]], 0)          # [T, D]
        # [128, NTT, NKC, 512]: xT[p, tt, kc, j] = xc[tt*512+j, kc*128+p]
        xT = np.ascontiguousarray(
            xc.reshape(NTT, 512, NKC, 128).transpose(3, 0, 2, 1)).astype(bf16)

        def wslice(name, scale=1.0):
            # [128, 2, NKC, 512]: w[p, s, kc, f] = w_s[kc*128+p, hsl.start+f]
            out = np.empty((128, 2, NKC, HPC * DH), bf16)
            for s in range(2):
                w = np.asarray(inputs[name + str(s + 1)], f32)[:, hsl] * scale
                out[:, s] = w.reshape(NKC, 128, HPC * DH).transpose(1, 0, 2).astype(bf16)
            return out

        wq = wslice("wq", SCALE)
        wk = wslice("wk")
        wv = wslice("wv")
        wo = np.empty((2, HPC, 128, D), bf16)
        for s in range(2):
            wo[s] = np.asarray(inputs["wo" + str(s + 1)], f32)[hsl, :].astype(bf16).reshape(HPC, 128, D)

        bias = np.zeros((128, 17), f32)
        bias[:, 16] = np.where(np.arange(128) % 2 == 0, -1.0, 1.0)
        for s in range(2):
            bqs = np.asarray(inputs["bq" + str(s + 1)], f32)[hsl] * SCALE
            bks = np.asarray(inputs["bk" + str(s + 1)], f32)[hsl]
            for h in range(HPC):
                bias[:, s * 8 + h] = bqs[h * DH:(h + 1) * DH]
                bias[:, s * 8 + 4 + h] = bks[h * DH:(h + 1) * DH]
        # bvb[p, s, plane, f] = bv_s[f]  (broadcast over partitions/planes)
        bvb = np.empty((128, 2, 2, 512), bf16)
        for s in range(2):
            bvs = np.asarray(inputs["bv" + str(s + 1)], f32)[hsl].astype(bf16)
            bvb[:, s] = np.broadcast_to(bvs, (128, 2, 512))

        in_maps.append({
            "xT": xT, "wq": wq, "wk": wk, "wv": wv, "wo": wo,
            "bias_qk": bias, "bvb": bvb, "cosT": cosT, "sinT": sinT, "Rt": rt,
        })
    return in_maps


def unshard_outputs(results, inputs):
    f32 = np.float32
    acc = np.zeros((B, D, T), f32)
    for c in range(N_CORES):
        # outT [128, NTT, NKC, 512] -> [D, T]: out[od*128+p, qt*512+j]
        r = results[c]["outT"].astype(f32)
        acc[c // 4] += r.transpose(2, 0, 1, 3).reshape(D, T)
    o1 = np.empty((B, N1, D), f32)
    o2 = np.empty((B, N2, D), f32)
    bo1 = np.asarray(inputs["bo1"], f32)
    bo2 = np.asarray(inputs["bo2"], f32)
    for b in range(B):
        full = acc[b].T                                  # [T, D]
        o1[b] = full[:N1] + bo1
        o2[b] = full[N1:] + bo2
    return o1, o2


def kernel(**inputs):
    nc = build_program()
    in_maps = shard_inputs(inputs)
    res = run_bass_kernel_spmd(nc, in_maps, list(range(N_CORES)))
    return unshard_outputs(res.results, inputs)


if __name__ == "__main__":
    data = np.load("/root/problem/cache_inputs.npz")
    out = kernel(**{k: data[k] for k in data.files})
    exp = np.load("/root/problem/cache_expected.npz")
    for i, o in enumerate(out):
        e = exp[f"o{i+1}"]
        d = np.abs(o - e).max()
        print(f"o{i+1}: absmax_err {d:.4e} rel {d / np.abs(e).max():.4e}")



# revision 52
# speedup vs baseline: 1.2069x; 1.0336x over previous
"""Dual-stream multi-head attention on 8 Trainium2 NeuronCores (Bass/Tile).

Sharding: core c handles batch b = c//4 and head-group g = c%4 (4 of 16 heads).
Each core computes QKV projections (per-stream weights), RoPE, joint attention
over both streams, and a partial output projection (its heads' rows of wo).
The host sums the 4 per-core partials of each batch (bf16 partials, fp32
accumulate), transposes, and adds the output bias.

Layout: fully transposed on-chip (features on partitions, tokens on the free
dim).  Scores are computed as S^T = k_rope @ q_rope^T so the PV matmul
consumes exp(S^T) with v in natural [token, dh] layout.

This revision runs at ~90% PE occupancy with 1584 matmuls x 213ns -- within
~10us of the bf16 streaming floor.  Key structure:
- ALL inputs are laid out partition-major on the host ([128, ...] with
  contiguous bytes per partition) so every DMA runs at full pipe rate; the
  "k p f -> p k f" on-the-fly rearrange only reaches ~180 GB/s.
- ALL input DMAs ride the SP queue in exact global consumption order (the
  DMA pipe is one shared ~350 GB/s resource; a dma_start also costs ~660ns
  on the issuing engine's sequencer, so the Act queue carries none and
  reaches the attention exps immediately at phase-C start; wo rides SP).
- tt0 runs its v-projection first (1 weight chunk per 4 matmuls matches the
  piece-DMA delivery rate), with 2-chunk x/wv pieces: PE starts ~2.5us in;
  a short dummy-matmul warmup before that keeps the HAM clock-gate hot.
- tt0's q/k runs chunk-major over head PAIRS (2 psum tiles) so its weight
  consumption (852ns/chunk) never outruns the wq/wk piece stream.
- v-bias applied via one broadcast-tile DVE add per v psum (not per-plane
  ones-matmuls); softmax denominators fold the two bf16 sacc planes on the
  DVE then use ONE ones-matmul per head (was two).
- q+k share a [128,2,512] psum pair per head; RoPE rotation stays on the PE
  (2 matmuls/head/tile; SBUF partition-pair-swap DMAs measured ~30us SLOWER
  on real HW than the sim claims); the rotation psum is read directly by
  the DVE sin-multiply (no Act copy).
- exp'd score psums in 2-bank pairs, one Act instruction per 1024 elements;
  output projection interleaved 2-matmuls-per-pair-step into the next
  query-tile's attention; output as bf16 pairs, one DMA per 2 output-dim
  chunks on the SP queue.
"""

import sys
import numpy as np

sys.path.insert(0, "/opt/trn_rl_repo")

import ml_dtypes
import concourse.bass as bass
import concourse.mybir as mybir
import concourse.tile as tile
from concourse.bass_utils import run_bass_kernel_spmd
from contextlib import ExitStack

B, N1, N2, D, H = 2, 1024, 1024, 2048, 16
T = N1 + N2              # 2048 tokens (both streams, concatenated)
DH = D // H              # 128
HPC = 4                  # heads per core
NKC = D // 128           # 16 contraction chunks
NTT = T // 512           # 4 512-token tiles
SCALE = DH ** -0.5
N_CORES = 8

BF = mybir.dt.bfloat16
F32 = mybir.dt.float32
bf16 = ml_dtypes.bfloat16
AF = mybir.ActivationFunctionType
ALU = mybir.AluOpType

_BUILT = {}


def build_program(repeats=1, phases="ABCD"):
    global _BUILT
    key = (repeats, phases)
    if key in _BUILT:
        return _BUILT[key]

    nc = bass.Bass()

    # all big tensors are laid out partition-major on the host so every DMA
    # reads/writes contiguous bytes per partition (the "k p f -> p k f"
    # rearrange pattern only reaches ~180 GB/s; these hit full rate)
    xT_d = nc.dram_tensor("xT", [128, NTT, NKC, 512], BF, kind="ExternalInput")
    wq_d = nc.dram_tensor("wq", [128, 2, NKC, HPC * DH], BF, kind="ExternalInput")
    wk_d = nc.dram_tensor("wk", [128, 2, NKC, HPC * DH], BF, kind="ExternalInput")
    wv_d = nc.dram_tensor("wv", [128, 2, NKC, HPC * DH], BF, kind="ExternalInput")
    wo_d = nc.dram_tensor("wo", [2, HPC, 128, D], BF, kind="ExternalInput")
    bias_d = nc.dram_tensor("bias_qk", [128, 17], F32, kind="ExternalInput")
    bvb_d = nc.dram_tensor("bvb", [128, 2, 2, 512], BF, kind="ExternalInput")
    cos_d = nc.dram_tensor("cosT", [128, T], BF, kind="ExternalInput")
    sin_d = nc.dram_tensor("sinT", [128, T], BF, kind="ExternalInput")
    rt_d = nc.dram_tensor("Rt", [128, 128], BF, kind="ExternalInput")
    out_d = nc.dram_tensor("outT", [128, NTT, NKC, 512], BF, kind="ExternalOutput")

    with tile.TileContext(nc) as tc:
        for _ in range(repeats):
            _emit(tc, nc, xT_d, wq_d, wk_d, wv_d, wo_d, bias_d, bvb_d, cos_d,
                  sin_d, rt_d, out_d, phases=phases)

    _split_dma_waits(nc)
    _BUILT[key] = nc
    return nc


def _split_dma_waits(nc):
    """This walrus build's 64-byte instruction encoding holds exactly one sync
    wait; peel extras into standalone EventSemaphore waits on the same
    engine immediately before the instruction."""
    wid = 0
    fn = nc.m.functions[0]
    for blk in fn.blocks:
        insts = blk.instructions
        out = []
        changed = False
        for inst in insts:
            si = inst.sync_info
            if si is not None and len(si.on_wait) > 1:
                waits = list(si.on_wait)
                for w in waits[:-1]:
                    pre = mybir.InstEventSemaphore(
                        name=f"WSPLIT-{wid}", ins=[], outs=[])
                    wid += 1
                    pre.engine = inst.engine
                    pre.sync_info = mybir.SyncInfo(on_wait=[w], on_update=[])
                    nc.register_instruction(pre, overwrite=True)
                    out.append(pre)
                inst.sync_info = mybir.SyncInfo(
                    on_wait=[waits[-1]], on_update=list(si.on_update))
                changed = True
            out.append(inst)
        if changed:
            blk.instructions = out


def _emit(tc, nc, xT_d, wq_d, wk_d, wv_d, wo_d, bias_d, bvb_d, cos_d, sin_d,
          rt_d, out_d, phases="ABCD"):
    with ExitStack() as top:
        consts = top.enter_context(tc.tile_pool(name="consts", bufs=1))
        persist = top.enter_context(tc.tile_pool(name="persist", bufs=1))

        rt_t = consts.tile([128, 128], BF, name="rt_t", tag="rt_t")
        bias_t = consts.tile([128, 17], F32, name="bias_t", tag="bias_t")
        bvb_t = consts.tile([128, 2, 2, 512], BF, name="bvb_t", tag="bvb_t")
        ones_t = consts.tile([128, 128], BF, name="ones_t", tag="ones_t")
        nc.vector.memset(ones_t[:], 1.0)
        zero_t = consts.tile([128, 1], F32, name="zero_t", tag="zero_t")
        nc.vector.memset(zero_t[:], 0.0)
        cosT = consts.tile([128, T], BF, name="cosT_t", tag="cosT_t")
        sinT = consts.tile([128, T], BF, name="sinT_t", tag="sinT_t")

        # qk_rope[h]: plane 0 = q_rope, plane 1 = k_rope  (bf16, [128, 2, T])
        qk_rope = [persist.tile([128, 2, T], BF, name=f"qkr{h}", tag=f"qkr{h}")
                   for h in range(HPC)]
        # v pairs: v_pair[p] covers token chunks (2p, 2p+1), natural layout
        v_pair = [persist.tile([128, 2, HPC * DH], BF, name=f"vp{p}", tag=f"vp{p}")
                  for p in range(NKC // 2)]

        # ---------------- Phase A: q,k projections + RoPE ------------------
        # ---------------- Phase B: v (natural layout) ----------------------
        with ExitStack() as ab:
            # per-stream full weight tiles [128, 16, 512] (tags carry s so
            # stream-1 DMAs prefetch during stream 0 without WAR waits); wv
            # reuses one tag (its s=1 DMA WAR-waits v(tt1)).  Stream-0 wq/wk
            # and the tt0 x tile are delivered as 2-chunk pieces interleaved
            # in consumption order so PE starts ~1us in.
            w_pool = ab.enter_context(tc.tile_pool(name="w", bufs=1))
            x_pool = ab.enter_context(tc.tile_pool(name="xs", bufs=2))
            x0_pool = ab.enter_context(tc.tile_pool(name="x0", bufs=1))
            sb_pool = ab.enter_context(tc.tile_pool(name="sb", bufs=2))
            t_pool = ab.enter_context(tc.tile_pool(name="tt", bufs=2))
            # one psum pool, 4 two-bank tags: qkp0/qkp1 alternate across
            # heads; rv0/rv1 are shared by the rotation psums (qk block) and
            # the v psums (v block) whose lifetimes interleave
            a_ps = ab.enter_context(tc.tile_pool(name="aps", bufs=1, space="PSUM"))

            HC = NKC // 2    # chunks per half-slab

            xt = {}          # tt -> (lo, hi) tiles [128, 8, 512]; tt=0: pieces
            x0p = []         # tt0 as 8 [128, 2, 512] piece tiles

            def xsl(tt, kc, toff=0, tn=512):
                if tt == 0:
                    return x0p[kc // 2][:, kc % 2, toff:toff + tn]
                return xt[tt][kc // HC][:, kc % HC, toff:toff + tn]

            def load_x(tt):
                # tt != 0 only (tt0 pieces are interleaved in the s==0 block)
                halves = []
                for i, nmi in ((0, "lo"), (1, "hi")):
                    t = x_pool.tile([128, HC, 512], BF, name=f"x{tt}{nmi}", tag=f"x{nmi}")
                    nc.sync.dma_start(t[:], xT_d[:, tt, i * HC:(i + 1) * HC, :])
                    halves.append(t)
                xt[tt] = halves

            def load_w(dram, s, tag, tag_s=True):
                t = w_pool.tile([128, NKC, HPC * DH], BF, name=f"{tag}{s}",
                                tag=f"{tag}{s}" if tag_s else tag)
                for i in range(2):
                    nc.sync.dma_start(t[:, i * HC:(i + 1) * HC, :],
                                      dram[:, s, i * HC:(i + 1) * HC, :])
                return t

            def emit_v(tt, s):
                for pl in range(2):
                    vp = tt * 2 + pl
                    vps = a_ps.tile([128, 2, 512], F32, name=f"vps{vp}", tag=f"rv{pl}")
                    for plane in range(2):
                        toff = pl * 256 + plane * 128
                        for kc in range(NKC):
                            nc.tensor.matmul(
                                vps[:, plane, :], xsl(tt, kc, toff, 128),
                                wv_t[s][:, kc, :],
                                start=(kc == 0), stop=(kc == NKC - 1))
                    # + bv (broadcast tile), psum -> sbuf on DVE
                    nc.vector.tensor_tensor(v_pair[vp][:], vps[:], bvb_t[:, s],
                                            ALU.add)

            def rope_tail(tt, s, h, qkp_or_sb):
                tsl = slice(tt * 512, (tt + 1) * 512)
                qkp = qkp_or_sb
                # sb2: plane 0 = q+bias, plane 1 = k+bias (contiguous)
                sb2 = sb_pool.tile([128, 2, 512], BF, name=f"sb2{tt}{h}", tag="sb2")
                nc.scalar.activation(sb2[:, 0, :], qkp[:, 0, :], AF.Identity,
                                     bias=bias_t[:, s * 8 + h:s * 8 + h + 1])
                nc.scalar.activation(sb2[:, 1, :], qkp[:, 1, :], AF.Identity,
                                     bias=bias_t[:, s * 8 + 4 + h:s * 8 + 4 + h + 1])
                sw = a_ps.tile([128, 2, 512], F32, name=f"rp{tt}{h}", tag=f"rv{h % 2}")
                nc.tensor.matmul(sw[:, 0, :], rt_t[:], sb2[:, 0, :], start=True, stop=True)
                nc.tensor.matmul(sw[:, 1, :], rt_t[:], sb2[:, 1, :], start=True, stop=True)
                t12 = t_pool.tile([128, 2, 512], BF, name=f"t12_{tt}{h}", tag="t12")
                nc.vector.tensor_tensor(t12[:, 0, :], sb2[:, 0, :], cosT[:, tsl], ALU.mult)
                nc.vector.tensor_tensor(t12[:, 1, :], sb2[:, 1, :], cosT[:, tsl], ALU.mult)
                t3 = t_pool.tile([128, 2, 512], BF, name=f"t3_{tt}{h}", tag="t3")
                nc.vector.tensor_tensor(t3[:, 0, :], sw[:, 0, :], sinT[:, tsl], ALU.mult)
                nc.vector.tensor_tensor(t3[:, 1, :], sw[:, 1, :], sinT[:, tsl], ALU.mult)
                nc.vector.tensor_tensor(qk_rope[h][:, :, tsl], t12[:], t3[:], ALU.add)

            def emit_qk(tt, s):
                for h in range(HPC):
                    hsl = slice(h * DH, (h + 1) * DH)
                    qkp = a_ps.tile([128, 2, 512], F32, name=f"qkp{tt}{h}",
                                    tag=f"qkp{h % 2}")
                    for kc in range(NKC):
                        nc.tensor.matmul(qkp[:, 0, :], wq_t[s][:, kc, hsl],
                                         xsl(tt, kc),
                                         start=(kc == 0), stop=(kc == NKC - 1))
                        nc.tensor.matmul(qkp[:, 1, :], wk_t[s][:, kc, hsl],
                                         xsl(tt, kc),
                                         start=(kc == 0), stop=(kc == NKC - 1))
                    rope_tail(tt, s, h, qkp)

            def emit_qk_pairs(tt, s):
                # chunk-major over head pairs: consumes each weight chunk at
                # 852ns vs the ~730ns/chunk DMA pipe delivery, so tt0's q/k
                # never outruns the interleaved wq/wk piece stream.
                for hp in range(2):
                    pair = (2 * hp, 2 * hp + 1)
                    qkps = [a_ps.tile([128, 2, 512], F32, name=f"qkp{tt}{h}",
                                       tag=f"qkp{h % 2}") for h in pair]
                    for kc in range(NKC):
                        for i, h in enumerate(pair):
                            hsl = slice(h * DH, (h + 1) * DH)
                            nc.tensor.matmul(qkps[i][:, 0, :], wq_t[s][:, kc, hsl],
                                             xsl(tt, kc),
                                             start=(kc == 0), stop=(kc == NKC - 1))
                            nc.tensor.matmul(qkps[i][:, 1, :], wk_t[s][:, kc, hsl],
                                             xsl(tt, kc),
                                             start=(kc == 0), stop=(kc == NKC - 1))
                    for i, h in enumerate(pair):
                        rope_tail(tt, s, h, qkps[i])

            # PE warmup: dummy matmuls from ~0.4us until the first v-chunk
            # DMAs land (~2.6us) keep the HAM activity window hot so real
            # matmuls start at full clock.
            wps = a_ps.tile([128, 512], F32, name="warm", tag="qkp0")
            for _ in range(20):
                nc.tensor.matmul(wps[:, 0:128], ones_t[:], ones_t[:],
                                 start=True, stop=True)

            wq_t, wk_t, wv_t = {}, {}, {}
            for s in range(2):
                if s == 0:
                    # ALL input DMAs ride ONE queue (SP) in exact global
                    # consumption order — the DMA pipe is a single shared
                    # ~350 GB/s resource, so cross-queue interleaving only
                    # scrambles the arrival order.  tt0 runs its
                    # v-projection FIRST (consumes 1 chunk per 4 matmuls =
                    # 852ns vs 730ns/chunk delivery) while wq/wk stream in
                    # behind wv; the Act queue carries no DMAs at all (a
                    # dma_start costs ~660ns on the issuing sequencer).
                    wv_t[0] = w_pool.tile([128, NKC, HPC * DH], BF, name="wv0",
                                          tag="wv")
                    for pc in range(8):
                        csl = slice(2 * pc, 2 * pc + 2)
                        t = x0_pool.tile([128, 2, 512], BF, name=f"x0p{pc}",
                                         tag=f"x0p{pc}")
                        nc.sync.dma_start(t[:], xT_d[:, 0, csl, :])
                        x0p.append(t)
                        nc.sync.dma_start(wv_t[0][:, csl, :], wv_d[:, 0, csl, :])
                    nc.sync.dma_start(bvb_t[:], bvb_d[:])
                    nc.sync.dma_start(rt_t[:], rt_d[:])
                    nc.sync.dma_start(bias_t[:], bias_d[:])
                    # wq/wk interleaved 2-chunk pieces in tt0's kc
                    # consumption order
                    wq_t[0] = w_pool.tile([128, NKC, HPC * DH], BF, name="wq0",
                                          tag="wq0")
                    wk_t[0] = w_pool.tile([128, NKC, HPC * DH], BF, name="wk0",
                                          tag="wk0")
                    for pc in range(8):
                        csl = slice(2 * pc, 2 * pc + 2)
                        nc.sync.dma_start(wq_t[0][:, csl, :], wq_d[:, 0, csl, :])
                        nc.sync.dma_start(wk_t[0][:, csl, :], wk_d[:, 0, csl, :])
                    nc.sync.dma_start(cosT[:], cos_d[:])
                    nc.sync.dma_start(sinT[:], sin_d[:])
                else:
                    wq_t[1] = load_w(wq_d, 1, "wq")
                    wk_t[1] = load_w(wk_d, 1, "wk")

                for tt in (2 * s, 2 * s + 1):
                    if tt != 0:
                        load_x(tt)
                    if tt == 2:
                        wv_t[1] = load_w(wv_d, 1, "wv", tag_s=False)
                    if tt == 0:
                        emit_v(0, 0)
                        emit_qk_pairs(0, 0)
                    else:
                        emit_qk(tt, s)
                        emit_v(tt, s)

        # ------- Phase C+D: attention + output projection, interleaved -----
        with ExitStack() as att:
            sps_ps = att.enter_context(tc.tile_pool(name="spsps", bufs=2, space="PSUM"))
            oacc_ps = att.enter_context(tc.tile_pool(name="oaccps", bufs=1, space="PSUM"))
            sums_ps = att.enter_context(tc.tile_pool(name="sumsps", bufs=1, space="PSUM"))
            out_ps = att.enter_context(tc.tile_pool(name="outps", bufs=2, space="PSUM"))
            es_pool = att.enter_context(tc.tile_pool(name="es", bufs=6))
            sacc_pool = att.enter_context(tc.tile_pool(name="sacc", bufs=2))
            sc2_pool = att.enter_context(tc.tile_pool(name="sc2", bufs=2))
            rc_pool = att.enter_context(tc.tile_pool(name="rc", bufs=2))
            on_pool = att.enter_context(tc.tile_pool(name="onorm", bufs=2))
            osb_pool = att.enter_context(tc.tile_pool(name="osb", bufs=2))
            wo_pool = att.enter_context(tc.tile_pool(name="wopool", bufs=1))

            # wo rides the SP queue: a dma_start costs ~660ns on the issuing
            # engine's sequencer, and the Act queue must reach the first exp
            # activations immediately at phase start.
            wo_t = [wo_pool.tile([128, HPC * D], BF, name=f"wos{s}", tag=f"wo{s}")
                    for s in range(2)]
            for s in range(2):
                for hd in range(HPC):
                    nc.sync.dma_start(wo_t[s][:, hd * D:(hd + 1) * D], wo_d[s, hd])

            onorm = {}

            def outproj_emitters(qt, tail=False):
                """One closure per output-projection MATMUL for query tile qt
                (4 per od-group); the psum copy + DMA ride on the 4th.  In the
                tail (last qt, nothing left to interleave with) rotate the od
                psums across the three same-sized psum pools so the Pool
                copies overlap the next groups' matmuls."""
                s = 0 if qt < 2 else 1
                qsl = slice(qt * 512, (qt + 1) * 512)
                pools = [(out_ps, "ops")]
                cell = {}
                items = []
                for od in range(NKC):
                    for hd in range(HPC):
                        def emit(od=od, hd=hd):
                            if hd == 0:
                                pool, tag = pools[od % len(pools)]
                                cell["ops"] = pool.tile(
                                    [128, 512], F32, name=f"op{qt}_{od}", tag=tag)
                            ops = cell["ops"]
                            nc.tensor.matmul(
                                ops[:],
                                wo_t[s][:, hd * D + od * 128: hd * D + (od + 1) * 128],
                                onorm[(hd, qt)][:],
                                start=(hd == 0), stop=(hd == HPC - 1))
                            if hd == HPC - 1:
                                if od % 4 == 0:
                                    cell["osb"] = osb_pool.tile(
                                        [128, 4, 512], BF, name=f"ou{qt}_{od}", tag="osb")
                                osb = cell["osb"]
                                nc.vector.tensor_copy(osb[:, od % 4, :], ops[:])
                                if od % 2 == 1:
                                    nc.sync.dma_start(
                                        out_d[:, qt, od - 1:od + 1, :],
                                        osb[:, (od % 4) - 1:(od % 4) + 1, :])
                        items.append(emit)
                return items

            pending = []
            for qt in range(NTT):
                qsl = slice(qt * 512, (qt + 1) * 512)
                for h in range(HPC):
                    hsl = slice(h * DH, (h + 1) * DH)
                    oacc = oacc_ps.tile([128, 512], F32, name=f"oa{h}{qt}", tag="oacc")
                    sacc = sacc_pool.tile([128, 2, 512], BF, name=f"sa{h}{qt}", tag="sacc")
                    for p in range(NKC // 2):
                        sps = sps_ps.tile([128, 2, 512], F32, name=f"sp{h}{qt}{p}", tag="sps")
                        for plane in range(2):
                            ksl = slice((2 * p + plane) * 128, (2 * p + plane + 1) * 128)
                            nc.tensor.matmul(sps[:, plane, :], qk_rope[h][:, 1, ksl],
                                             qk_rope[h][:, 0, qsl], start=True, stop=True)
                        es = es_pool.tile([128, 2, 512], BF, name=f"es{h}{qt}{p}", tag="es")
                        nc.scalar.activation(es[:], sps[:], AF.Exp, bias=zero_t[:, 0:1])
                        for plane in range(2):
                            nc.tensor.matmul(oacc[:], v_pair[p][:, plane, hsl],
                                             es[:, plane, :],
                                             start=(p == 0 and plane == 0),
                                             stop=(p == NKC // 2 - 1 and plane == 1))
                        if p == 0:
                            nc.vector.tensor_copy(sacc[:], es[:])
                        else:
                            nc.vector.tensor_tensor(sacc[:], sacc[:], es[:], ALU.add)
                        for _ in range(2):
                            if pending:
                                pending.pop(0)()
                    sc2 = sc2_pool.tile([128, 512], BF, name=f"sc{h}{qt}", tag="sc2")
                    nc.vector.tensor_tensor(sc2[:], sacc[:, 0, :], sacc[:, 1, :], ALU.add)
                    sums = sums_ps.tile([128, 512], F32, name=f"su{h}{qt}", tag="sums")
                    nc.tensor.matmul(sums[:], ones_t[:], sc2[:], start=True, stop=True)
                    rc = rc_pool.tile([128, 512], F32, name=f"rc{h}{qt}", tag="rc")
                    nc.vector.reciprocal(rc[:], sums[:])
                    on_t = on_pool.tile([128, 512], BF, name=f"on{h}{qt}", tag=f"on{h}")
                    nc.vector.tensor_tensor(on_t[:], oacc[:], rc[:], ALU.mult)
                    onorm[(h, qt)] = on_t
                while pending:
                    pending.pop(0)()
                pending = outproj_emitters(qt, tail=(qt == NTT - 1))
            while pending:
                pending.pop(0)()


def shard_inputs(inputs):
    """Full inputs -> per-core in_maps (host-side prep: transpose, cast,
    scale-folding, per-head slicing)."""
    f32 = np.float32
    x1, x2 = np.asarray(inputs["x_1"], f32), np.asarray(inputs["x_2"], f32)
    cosT = np.ascontiguousarray(
        np.concatenate([np.asarray(inputs["cos1"]), np.asarray(inputs["cos2"])], 0).T
    ).astype(bf16)
    sinT = np.ascontiguousarray(
        np.concatenate([np.asarray(inputs["sin1"]), np.asarray(inputs["sin2"])], 0).T
    ).astype(bf16)
    rt = np.zeros((128, 128), np.float32)
    idx = np.arange(0, 128, 2)
    rt[idx, idx + 1] = 1.0
    rt[idx + 1, idx] = -1.0
    rt = rt.astype(bf16)

    in_maps = []
    for c in range(N_CORES):
        b, hg = divmod(c, 4)
        hsl = slice(hg * HPC * DH, (hg + 1) * HPC * DH)
        xc = np.concatenate([x1[b], x2[b]], 0)          # [T, D]
        # [128, NTT, NKC, 512]: xT[p, tt, kc, j] = xc[tt*512+j, kc*128+p]
        xT = np.ascontiguousarray(
            xc.reshape(NTT, 512, NKC, 128).transpose(3, 0, 2, 1)).astype(bf16)

        def wslice(name, scale=1.0):
            # [128, 2, NKC, 512]: w[p, s, kc, f] = w_s[kc*128+p, hsl.start+f]
            out = np.empty((128, 2, NKC, HPC * DH), bf16)
            for s in range(2):
                w = np.asarray(inputs[name + str(s + 1)], f32)[:, hsl] * scale
                out[:, s] = w.reshape(NKC, 128, HPC * DH).transpose(1, 0, 2).astype(bf16)
            return out

        wq = wslice("wq", SCALE)
        wk = wslice("wk")
        wv = wslice("wv")
        wo = np.empty((2, HPC, 128, D), bf16)
        for s in range(2):
            wo[s] = np.asarray(inputs["wo" + str(s + 1)], f32)[hsl, :].astype(bf16).reshape(HPC, 128, D)

        bias = np.zeros((128, 17), f32)
        bias[:, 16] = np.where(np.arange(128) % 2 == 0, -1.0, 1.0)
        for s in range(2):
            bqs = np.asarray(inputs["bq" + str(s + 1)], f32)[hsl] * SCALE
            bks = np.asarray(inputs["bk" + str(s + 1)], f32)[hsl]
            for h in range(HPC):
                bias[:, s * 8 + h] = bqs[h * DH:(h + 1) * DH]
                bias[:, s * 8 + 4 + h] = bks[h * DH:(h + 1) * DH]
        # bvb[p, s, plane, f] = bv_s[f]  (broadcast over partitions/planes)
        bvb = np.empty((128, 2, 2, 512), bf16)
        for s in range(2):
            bvs = np.asarray(inputs["bv" + str(s + 1)], f32)[hsl].astype(bf16)
            bvb[:, s] = np.broadcast_to(bvs, (128, 2, 512))

        in_maps.append({
            "xT": xT, "wq": wq, "wk": wk, "wv": wv, "wo": wo,
            "bias_qk": bias, "bvb": bvb, "cosT": cosT, "sinT": sinT, "Rt": rt,
        })
    return in_maps


def unshard_outputs(results, inputs):
    f32 = np.float32
    acc = np.zeros((B, D, T), f32)
    for c in range(N_CORES):
        # outT [128, NTT, NKC, 512] -> [D, T]: out[od*128+p, qt*512+j]
        r = results[c]["outT"].astype(f32)
        acc[c // 4] += r.transpose(2, 0, 1, 3).reshape(D, T)
    o1 = np.empty((B, N1, D), f32)
    o2 = np.empty((B, N2, D), f32)
    bo1 = np.asarray(inputs["bo1"], f32)
    bo2 = np.asarray(inputs["bo2"], f32)
    for b in range(B):
        full = acc[b].T                                  # [T, D]
        o1[b] = full[:N1] + bo1
        o2[b] = full[N1:] + bo2
    return o1, o2


def kernel(**inputs):
    nc = build_program()
    in_maps = shard_inputs(inputs)
    res = run_bass_kernel_spmd(nc, in_maps, list(range(N_CORES)))
    return unshard_outputs(res.results, inputs)


if __name__ == "__main__":
    data = np.load("/root/problem/cache_inputs.npz")
    out = kernel(**{k: data[k] for k in data.files})
    exp = np.load("/root/problem/cache_expected.npz")
    for i, o in enumerate(out):
        e = exp[f"o{i+1}"]
        d = np.abs(o - e).max()
        print(f"o{i+1}: absmax_err {d:.4e} rel {d / np.abs(e).max():.4e}")



# revision 55
# speedup vs baseline: 1.2655x; 1.0486x over previous
"""Dual-stream multi-head attention on 8 Trainium2 NeuronCores (Bass/Tile).

Sharding: core c handles batch b = c//4 and head-group g = c%4 (4 of 16 heads).
Each core computes QKV projections (per-stream weights), RoPE, joint attention
over both streams, and a partial output projection (its heads' rows of wo).
The host sums the 4 per-core partials of each batch (bf16 partials, fp32
accumulate), transposes, and adds the output bias.

Layout: fully transposed on-chip (features on partitions, tokens on the free
dim).  Scores are computed as S^T = k_rope @ q_rope^T so the PV matmul
consumes exp(S^T) with v in natural [token, dh] layout.

This revision runs at ~90% PE occupancy with 1584 matmuls x 213ns -- within
~10us of the bf16 streaming floor.  Key structure:
- ALL inputs are laid out partition-major on the host ([128, ...] with
  contiguous bytes per partition) so every DMA runs at full pipe rate; the
  "k p f -> p k f" on-the-fly rearrange only reaches ~180 GB/s.
- ALL input DMAs ride the SP queue in exact global consumption order (the
  DMA pipe is one shared ~350 GB/s resource; a dma_start also costs ~660ns
  on the issuing engine's sequencer, so the Act queue carries none and
  reaches the attention exps immediately at phase-C start; wo rides SP).
- tt0 runs its v-projection first (1 weight chunk per 4 matmuls matches the
  piece-DMA delivery rate), with 2-chunk x/wv pieces: PE starts ~2.5us in;
  a short dummy-matmul warmup before that keeps the HAM clock-gate hot.
- tt0's q/k runs chunk-major over head PAIRS (2 psum tiles) so its weight
  consumption (852ns/chunk) never outruns the wq/wk piece stream.
- v-bias applied via one broadcast-tile DVE add per v psum (not per-plane
  ones-matmuls); softmax denominators fold the two bf16 sacc planes on the
  DVE then use ONE ones-matmul per head (was two).
- q+k share a [128,2,512] psum pair per head; RoPE rotation stays on the PE
  (2 matmuls/head/tile; SBUF partition-pair-swap DMAs measured ~30us SLOWER
  on real HW than the sim claims); the rotation psum is read directly by
  the DVE sin-multiply (no Act copy).
- exp'd score psums in 2-bank pairs, one Act instruction per 1024 elements;
  output projection interleaved 2-matmuls-per-pair-step into the next
  query-tile's attention; output as bf16 pairs, one DMA per 2 output-dim
  chunks on the SP queue.
"""

import sys
import numpy as np

sys.path.insert(0, "/opt/trn_rl_repo")

import ml_dtypes
import concourse.bass as bass
import concourse.mybir as mybir
import concourse.tile as tile
from concourse.bass_utils import run_bass_kernel_spmd
from contextlib import ExitStack

B, N1, N2, D, H = 2, 1024, 1024, 2048, 16
T = N1 + N2              # 2048 tokens (both streams, concatenated)
DH = D // H              # 128
HPC = 4                  # heads per core
NKC = D // 128           # 16 contraction chunks
NTT = T // 512           # 4 512-token tiles
SCALE = DH ** -0.5
N_CORES = 8

BF = mybir.dt.bfloat16
F32 = mybir.dt.float32
bf16 = ml_dtypes.bfloat16
AF = mybir.ActivationFunctionType
ALU = mybir.AluOpType

_BUILT = {}


def build_program(repeats=1, phases="ABCD"):
    global _BUILT
    key = (repeats, phases)
    if key in _BUILT:
        return _BUILT[key]

    nc = bass.Bass()

    # all big tensors are laid out partition-major on the host so every DMA
    # reads/writes contiguous bytes per partition (the "k p f -> p k f"
    # rearrange pattern only reaches ~180 GB/s; these hit full rate)
    xT_d = nc.dram_tensor("xT", [128, NTT, NKC, 512], BF, kind="ExternalInput")
    wq_d = nc.dram_tensor("wq", [128, 2, NKC, HPC * DH], BF, kind="ExternalInput")
    wk_d = nc.dram_tensor("wk", [128, 2, NKC, HPC * DH], BF, kind="ExternalInput")
    wv_d = nc.dram_tensor("wv", [128, 2, NKC, HPC * DH], BF, kind="ExternalInput")
    wo_d = nc.dram_tensor("wo", [2, HPC, 128, D], BF, kind="ExternalInput")
    bias_d = nc.dram_tensor("bias_qk", [128, 17], F32, kind="ExternalInput")
    bvb_d = nc.dram_tensor("bvb", [128, 2, 2, 512], BF, kind="ExternalInput")
    cos_d = nc.dram_tensor("cosT", [128, T], BF, kind="ExternalInput")
    sin_d = nc.dram_tensor("sinT", [128, T], BF, kind="ExternalInput")
    rt_d = nc.dram_tensor("Rt", [128, 128], BF, kind="ExternalInput")
    out_d = nc.dram_tensor("outT", [128, NTT, NKC, 512], BF, kind="ExternalOutput")

    with tile.TileContext(nc) as tc:
        for _ in range(repeats):
            _emit(tc, nc, xT_d, wq_d, wk_d, wv_d, wo_d, bias_d, bvb_d, cos_d,
                  sin_d, rt_d, out_d, phases=phases)

    _split_dma_waits(nc)
    _BUILT[key] = nc
    return nc


def _split_dma_waits(nc):
    """This walrus build's 64-byte instruction encoding holds exactly one sync
    wait; peel extras into standalone EventSemaphore waits on the same
    engine immediately before the instruction."""
    wid = 0
    fn = nc.m.functions[0]
    for blk in fn.blocks:
        insts = blk.instructions
        out = []
        changed = False
        for inst in insts:
            si = inst.sync_info
            if si is not None and len(si.on_wait) > 1:
                waits = list(si.on_wait)
                for w in waits[:-1]:
                    pre = mybir.InstEventSemaphore(
                        name=f"WSPLIT-{wid}", ins=[], outs=[])
                    wid += 1
                    pre.engine = inst.engine
                    pre.sync_info = mybir.SyncInfo(on_wait=[w], on_update=[])
                    nc.register_instruction(pre, overwrite=True)
                    out.append(pre)
                inst.sync_info = mybir.SyncInfo(
                    on_wait=[waits[-1]], on_update=list(si.on_update))
                changed = True
            out.append(inst)
        if changed:
            blk.instructions = out


def _emit(tc, nc, xT_d, wq_d, wk_d, wv_d, wo_d, bias_d, bvb_d, cos_d, sin_d,
          rt_d, out_d, phases="ABCD"):
    with ExitStack() as top:
        consts = top.enter_context(tc.tile_pool(name="consts", bufs=1))
        persist = top.enter_context(tc.tile_pool(name="persist", bufs=1))

        rt_t = consts.tile([128, 128], BF, name="rt_t", tag="rt_t")
        bias_t = consts.tile([128, 17], F32, name="bias_t", tag="bias_t")
        bvb_t = consts.tile([128, 2, 2, 512], BF, name="bvb_t", tag="bvb_t")
        ones_t = consts.tile([128, 128], BF, name="ones_t", tag="ones_t")
        nc.vector.memset(ones_t[:], 1.0)
        zero_t = consts.tile([128, 1], F32, name="zero_t", tag="zero_t")
        nc.vector.memset(zero_t[:], 0.0)
        cosT = consts.tile([128, T], BF, name="cosT_t", tag="cosT_t")
        sinT = consts.tile([128, T], BF, name="sinT_t", tag="sinT_t")

        # qk_rope[h]: plane 0 = q_rope, plane 1 = k_rope  (bf16, [128, 2, T])
        qk_rope = [persist.tile([128, 2, T], BF, name=f"qkr{h}", tag=f"qkr{h}")
                   for h in range(HPC)]
        # v pairs: v_pair[p] covers token chunks (2p, 2p+1), natural layout
        v_pair = [persist.tile([128, 2, HPC * DH], BF, name=f"vp{p}", tag=f"vp{p}")
                  for p in range(NKC // 2)]

        # ---------------- Phase A: q,k projections + RoPE ------------------
        # ---------------- Phase B: v (natural layout) ----------------------
        with ExitStack() as ab:
            # per-stream full weight tiles [128, 16, 512] (tags carry s so
            # stream-1 DMAs prefetch during stream 0 without WAR waits); wv
            # reuses one tag (its s=1 DMA WAR-waits v(tt1)).  Stream-0 wq/wk
            # and the tt0 x tile are delivered as 2-chunk pieces interleaved
            # in consumption order so PE starts ~1us in.
            w_pool = ab.enter_context(tc.tile_pool(name="w", bufs=1))
            x_pool = ab.enter_context(tc.tile_pool(name="xs", bufs=2))
            x0_pool = ab.enter_context(tc.tile_pool(name="x0", bufs=1))
            sb_pool = ab.enter_context(tc.tile_pool(name="sb", bufs=2))
            t_pool = ab.enter_context(tc.tile_pool(name="tt", bufs=2))
            # one psum pool, 4 two-bank tags: qkp0/qkp1 alternate across
            # heads; rv0/rv1 are shared by the rotation psums (qk block) and
            # the v psums (v block) whose lifetimes interleave
            a_ps = ab.enter_context(tc.tile_pool(name="aps", bufs=1, space="PSUM"))

            HC = NKC // 2    # chunks per half-slab

            xt = {}          # tt -> (lo, hi) tiles [128, 8, 512]; tt=0: pieces
            x0p = []         # tt0 as 8 [128, 2, 512] piece tiles

            def xsl(tt, kc, toff=0, tn=512):
                if tt == 0:
                    return x0p[kc // 2][:, kc % 2, toff:toff + tn]
                return xt[tt][kc // HC][:, kc % HC, toff:toff + tn]

            def load_x(tt):
                # tt != 0 only (tt0 pieces are interleaved in the s==0 block)
                halves = []
                for i, nmi in ((0, "lo"), (1, "hi")):
                    t = x_pool.tile([128, HC, 512], BF, name=f"x{tt}{nmi}", tag=f"x{nmi}")
                    nc.sync.dma_start(t[:], xT_d[:, tt, i * HC:(i + 1) * HC, :])
                    halves.append(t)
                xt[tt] = halves

            def load_w(dram, s, tag, tag_s=True):
                t = w_pool.tile([128, NKC, HPC * DH], BF, name=f"{tag}{s}",
                                tag=f"{tag}{s}" if tag_s else tag)
                for i in range(2):
                    nc.sync.dma_start(t[:, i * HC:(i + 1) * HC, :],
                                      dram[:, s, i * HC:(i + 1) * HC, :])
                return t

            def emit_v(tt, s):
                for pl in range(2):
                    vp = tt * 2 + pl
                    vps = a_ps.tile([128, 2, 512], F32, name=f"vps{vp}", tag=f"rv{pl}")
                    for plane in range(2):
                        toff = pl * 256 + plane * 128
                        for kc in range(NKC):
                            nc.tensor.matmul(
                                vps[:, plane, :], xsl(tt, kc, toff, 128),
                                wv_t[s][:, kc, :],
                                start=(kc == 0), stop=(kc == NKC - 1))
                    # + bv (broadcast tile), psum -> sbuf on DVE
                    nc.vector.tensor_tensor(v_pair[vp][:], vps[:], bvb_t[:, s],
                                            ALU.add)

            def rope_tail(tt, s, h, qkp_or_sb):
                tsl = slice(tt * 512, (tt + 1) * 512)
                qkp = qkp_or_sb
                # sb2: plane 0 = q+bias, plane 1 = k+bias (contiguous)
                sb2 = sb_pool.tile([128, 2, 512], BF, name=f"sb2{tt}{h}", tag="sb2")
                nc.scalar.activation(sb2[:, 0, :], qkp[:, 0, :], AF.Identity,
                                     bias=bias_t[:, s * 8 + h:s * 8 + h + 1])
                nc.scalar.activation(sb2[:, 1, :], qkp[:, 1, :], AF.Identity,
                                     bias=bias_t[:, s * 8 + 4 + h:s * 8 + 4 + h + 1])
                sw = a_ps.tile([128, 2, 512], F32, name=f"rp{tt}{h}", tag=f"rv{h % 2}")
                nc.tensor.matmul(sw[:, 0, :], rt_t[:], sb2[:, 0, :], start=True, stop=True)
                nc.tensor.matmul(sw[:, 1, :], rt_t[:], sb2[:, 1, :], start=True, stop=True)
                t12 = t_pool.tile([128, 2, 512], BF, name=f"t12_{tt}{h}", tag="t12")
                nc.vector.tensor_tensor(t12[:, 0, :], sb2[:, 0, :], cosT[:, tsl], ALU.mult)
                nc.vector.tensor_tensor(t12[:, 1, :], sb2[:, 1, :], cosT[:, tsl], ALU.mult)
                t3 = t_pool.tile([128, 2, 512], BF, name=f"t3_{tt}{h}", tag="t3")
                nc.vector.tensor_tensor(t3[:, 0, :], sw[:, 0, :], sinT[:, tsl], ALU.mult)
                nc.vector.tensor_tensor(t3[:, 1, :], sw[:, 1, :], sinT[:, tsl], ALU.mult)
                nc.vector.tensor_tensor(qk_rope[h][:, :, tsl], t12[:], t3[:], ALU.add)

            def emit_qk(tt, s):
                for h in range(HPC):
                    hsl = slice(h * DH, (h + 1) * DH)
                    qkp = a_ps.tile([128, 2, 512], F32, name=f"qkp{tt}{h}",
                                    tag=f"qkp{h % 2}")
                    for kc in range(NKC):
                        nc.tensor.matmul(qkp[:, 0, :], wq_t[s][:, kc, hsl],
                                         xsl(tt, kc),
                                         start=(kc == 0), stop=(kc == NKC - 1))
                        nc.tensor.matmul(qkp[:, 1, :], wk_t[s][:, kc, hsl],
                                         xsl(tt, kc),
                                         start=(kc == 0), stop=(kc == NKC - 1))
                    rope_tail(tt, s, h, qkp)

            def emit_qk_pairs(tt, s):
                # chunk-major over head pairs: consumes each weight chunk at
                # 852ns vs the ~730ns/chunk DMA pipe delivery, so tt0's q/k
                # never outruns the interleaved wq/wk piece stream.
                for hp in range(2):
                    pair = (2 * hp, 2 * hp + 1)
                    qkps = [a_ps.tile([128, 2, 512], F32, name=f"qkp{tt}{h}",
                                       tag=f"qkp{h % 2}") for h in pair]
                    for kc in range(NKC):
                        for i, h in enumerate(pair):
                            hsl = slice(h * DH, (h + 1) * DH)
                            nc.tensor.matmul(qkps[i][:, 0, :], wq_t[s][:, kc, hsl],
                                             xsl(tt, kc),
                                             start=(kc == 0), stop=(kc == NKC - 1))
                            nc.tensor.matmul(qkps[i][:, 1, :], wk_t[s][:, kc, hsl],
                                             xsl(tt, kc),
                                             start=(kc == 0), stop=(kc == NKC - 1))
                    for i, h in enumerate(pair):
                        rope_tail(tt, s, h, qkps[i])

            # PE warmup: dummy matmuls from ~0.4us until the first v-chunk
            # DMAs land (~2.6us) keep the HAM activity window hot so real
            # matmuls start at full clock.
            wps = a_ps.tile([128, 512], F32, name="warm", tag="qkp0")
            for _ in range(20):
                nc.tensor.matmul(wps[:, 0:128], ones_t[:], ones_t[:],
                                 start=True, stop=True)

            wq_t, wk_t, wv_t = {}, {}, {}
            for s in range(2):
                if s == 0:
                    # ALL input DMAs ride ONE queue (SP) in exact global
                    # consumption order — the DMA pipe is a single shared
                    # ~350 GB/s resource, so cross-queue interleaving only
                    # scrambles the arrival order.  tt0 runs its
                    # v-projection FIRST (consumes 1 chunk per 4 matmuls =
                    # 852ns vs 730ns/chunk delivery) while wq/wk stream in
                    # behind wv; the Act queue carries no DMAs at all (a
                    # dma_start costs ~660ns on the issuing sequencer).
                    wv_t[0] = w_pool.tile([128, NKC, HPC * DH], BF, name="wv0",
                                          tag="wv")
                    for pc in range(8):
                        csl = slice(2 * pc, 2 * pc + 2)
                        t = x0_pool.tile([128, 2, 512], BF, name=f"x0p{pc}",
                                         tag=f"x0p{pc}")
                        nc.sync.dma_start(t[:], xT_d[:, 0, csl, :])
                        x0p.append(t)
                        nc.sync.dma_start(wv_t[0][:, csl, :], wv_d[:, 0, csl, :])
                    nc.sync.dma_start(bvb_t[:], bvb_d[:])
                    nc.sync.dma_start(rt_t[:], rt_d[:])
                    nc.sync.dma_start(bias_t[:], bias_d[:])
                    # wq/wk interleaved 2-chunk pieces in tt0's kc
                    # consumption order
                    wq_t[0] = w_pool.tile([128, NKC, HPC * DH], BF, name="wq0",
                                          tag="wq0")
                    wk_t[0] = w_pool.tile([128, NKC, HPC * DH], BF, name="wk0",
                                          tag="wk0")
                    for pc in range(8):
                        csl = slice(2 * pc, 2 * pc + 2)
                        nc.sync.dma_start(wq_t[0][:, csl, :], wq_d[:, 0, csl, :])
                        nc.sync.dma_start(wk_t[0][:, csl, :], wk_d[:, 0, csl, :])
                    nc.sync.dma_start(cosT[:], cos_d[:])
                    nc.sync.dma_start(sinT[:], sin_d[:])
                else:
                    wq_t[1] = load_w(wq_d, 1, "wq")
                    wk_t[1] = load_w(wk_d, 1, "wk")

                for tt in (2 * s, 2 * s + 1):
                    if tt != 0:
                        load_x(tt)
                    if tt == 2:
                        wv_t[1] = load_w(wv_d, 1, "wv", tag_s=False)
                    if tt == 0:
                        emit_v(0, 0)
                        emit_qk_pairs(0, 0)
                    else:
                        emit_qk(tt, s)
                        emit_v(tt, s)

        # ------- Phase C+D: attention + output projection, interleaved -----
        with ExitStack() as att:
            sps_ps = att.enter_context(tc.tile_pool(name="spsps", bufs=2, space="PSUM"))
            oacc_ps = att.enter_context(tc.tile_pool(name="oaccps", bufs=1, space="PSUM"))
            sums_ps = att.enter_context(tc.tile_pool(name="sumsps", bufs=1, space="PSUM"))
            out_ps = att.enter_context(tc.tile_pool(name="outps", bufs=2, space="PSUM"))
            es_pool = att.enter_context(tc.tile_pool(name="es", bufs=6))
            sacc_pool = att.enter_context(tc.tile_pool(name="sacc", bufs=2))
            sc2_pool = att.enter_context(tc.tile_pool(name="sc2", bufs=2))
            rc_pool = att.enter_context(tc.tile_pool(name="rc", bufs=2))
            on_pool = att.enter_context(tc.tile_pool(name="onorm", bufs=2))
            osb_pool = att.enter_context(tc.tile_pool(name="osb", bufs=2))
            wo_pool = att.enter_context(tc.tile_pool(name="wopool", bufs=1))

            # wo rides the SP queue: a dma_start costs ~660ns on the issuing
            # engine's sequencer, and the Act queue must reach the first exp
            # activations immediately at phase start.
            wo_t = [wo_pool.tile([128, HPC * D], BF, name=f"wos{s}", tag=f"wo{s}")
                    for s in range(2)]
            for s in range(2):
                for hd in range(HPC):
                    nc.sync.dma_start(wo_t[s][:, hd * D:(hd + 1) * D], wo_d[s, hd])

            onorm = {}

            def outproj_emitters(qt, tail=False):
                """One closure per output-projection MATMUL for query tile qt
                (4 per od-group); the psum copy + DMA ride on the 4th.  In the
                tail (last qt, nothing left to interleave with) rotate the od
                psums across the three same-sized psum pools so the Pool
                copies overlap the next groups' matmuls."""
                s = 0 if qt < 2 else 1
                qsl = slice(qt * 512, (qt + 1) * 512)
                pools = [(out_ps, "ops")]
                cell = {}
                items = []
                for od in range(NKC):
                    for hd in range(HPC):
                        def emit(od=od, hd=hd):
                            if hd == 0:
                                pool, tag = pools[od % len(pools)]
                                cell["ops"] = pool.tile(
                                    [128, 512], F32, name=f"op{qt}_{od}", tag=tag)
                            ops = cell["ops"]
                            nc.tensor.matmul(
                                ops[:],
                                wo_t[s][:, hd * D + od * 128: hd * D + (od + 1) * 128],
                                onorm[(hd, qt)][:],
                                start=(hd == 0), stop=(hd == HPC - 1))
                            if hd == HPC - 1:
                                if od % 4 == 0:
                                    cell["osb"] = osb_pool.tile(
                                        [128, 4, 512], BF, name=f"ou{qt}_{od}", tag="osb")
                                osb = cell["osb"]
                                nc.vector.tensor_copy(osb[:, od % 4, :], ops[:])
                                if od % 2 == 1:
                                    nc.sync.dma_start(
                                        out_d[:, qt, od - 1:od + 1, :],
                                        osb[:, (od % 4) - 1:(od % 4) + 1, :])
                        items.append(emit)
                return items

            pending = []
            for qt in range(NTT):
                qsl = slice(qt * 512, (qt + 1) * 512)
                for h in range(HPC):
                    hsl = slice(h * DH, (h + 1) * DH)
                    oacc = oacc_ps.tile([128, 512], F32, name=f"oa{h}{qt}", tag="oacc")
                    sacc = sacc_pool.tile([128, 2, 512], BF, name=f"sa{h}{qt}", tag="sacc")
                    for p in range(NKC // 2):
                        sps = sps_ps.tile([128, 2, 512], F32, name=f"sp{h}{qt}{p}", tag="sps")
                        for plane in range(2):
                            ksl = slice((2 * p + plane) * 128, (2 * p + plane + 1) * 128)
                            nc.tensor.matmul(sps[:, plane, :], qk_rope[h][:, 1, ksl],
                                             qk_rope[h][:, 0, qsl], start=True, stop=True)
                        es = es_pool.tile([128, 2, 512], BF, name=f"es{h}{qt}{p}", tag="es")
                        nc.scalar.activation(es[:], sps[:], AF.Exp, bias=zero_t[:, 0:1])
                        for plane in range(2):
                            nc.tensor.matmul(oacc[:], v_pair[p][:, plane, hsl],
                                             es[:, plane, :],
                                             start=(p == 0 and plane == 0),
                                             stop=(p == NKC // 2 - 1 and plane == 1))
                        if p == 0:
                            nc.vector.tensor_copy(sacc[:], es[:])
                        else:
                            nc.vector.tensor_tensor(sacc[:], sacc[:], es[:], ALU.add)
                        for _ in range(2):
                            if pending:
                                pending.pop(0)()
                    sc2 = sc2_pool.tile([128, 512], BF, name=f"sc{h}{qt}", tag="sc2")
                    nc.vector.tensor_tensor(sc2[:], sacc[:, 0, :], sacc[:, 1, :], ALU.add)
                    sums = sums_ps.tile([128, 512], F32, name=f"su{h}{qt}", tag="sums")
                    nc.tensor.matmul(sums[:], ones_t[:], sc2[:], start=True, stop=True)
                    rc = rc_pool.tile([128, 512], F32, name=f"rc{h}{qt}", tag="rc")
                    nc.vector.reciprocal(rc[:], sums[:])
                    on_t = on_pool.tile([128, 512], BF, name=f"on{h}{qt}", tag=f"on{h}")
                    nc.vector.tensor_tensor(on_t[:], oacc[:], rc[:], ALU.mult)
                    onorm[(h, qt)] = on_t
                while pending:
                    pending.pop(0)()
                pending = outproj_emitters(qt, tail=(qt == NTT - 1))
            while pending:
                pending.pop(0)()


def shard_inputs(inputs):
    """Full inputs -> per-core in_maps (host-side prep: transpose, cast,
    scale-folding, per-head slicing)."""
    f32 = np.float32
    x1, x2 = np.asarray(inputs["x_1"], f32), np.asarray(inputs["x_2"], f32)
    cosT = np.ascontiguousarray(
        np.concatenate([np.asarray(inputs["cos1"]), np.asarray(inputs["cos2"])], 0).T
    ).astype(bf16)
    sinT = np.ascontiguousarray(
        np.concatenate([np.asarray(inputs["sin1"]), np.asarray(inputs["sin2"])], 0).T
    ).astype(bf16)
    rt = np.zeros((128, 128), np.float32)
    idx = np.arange(0, 128, 2)
    rt[idx, idx + 1] = 1.0
    rt[idx + 1, idx] = -1.0
    rt = rt.astype(bf16)

    in_maps = []
    for c in range(N_CORES):
        b, hg = divmod(c, 4)
        hsl = slice(hg * HPC * DH, (hg + 1) * HPC * DH)
        xc = np.concatenate([x1[b], x2[b]], 0)          # [T, D]
        # [128, NTT, NKC, 512]: xT[p, tt, kc, j] = xc[tt*512+j, kc*128+p]
        xT = np.ascontiguousarray(
            xc.reshape(NTT, 512, NKC, 128).transpose(3, 0, 2, 1)).astype(bf16)

        def wslice(name, scale=1.0):
            # [128, 2, NKC, 512]: w[p, s, kc, f] = w_s[kc*128+p, hsl.start+f]
            out = np.empty((128, 2, NKC, HPC * DH), bf16)
            for s in range(2):
                w = np.asarray(inputs[name + str(s + 1)], f32)[:, hsl] * scale
                out[:, s] = w.reshape(NKC, 128, HPC * DH).transpose(1, 0, 2).astype(bf16)
            return out

        wq = wslice("wq", SCALE)
        wk = wslice("wk")
        wv = wslice("wv")
        wo = np.empty((2, HPC, 128, D), bf16)
        for s in range(2):
            wo[s] = np.asarray(inputs["wo" + str(s + 1)], f32)[hsl, :].astype(bf16).reshape(HPC, 128, D)

        bias = np.zeros((128, 17), f32)
        bias[:, 16] = np.where(np.arange(128) % 2 == 0, -1.0, 1.0)
        for s in range(2):
            bqs = np.asarray(inputs["bq" + str(s + 1)], f32)[hsl] * SCALE
            bks = np.asarray(inputs["bk" + str(s + 1)], f32)[hsl]
            for h in range(HPC):
                bias[:, s * 8 + h] = bqs[h * DH:(h + 1) * DH]
                bias[:, s * 8 + 4 + h] = bks[h * DH:(h + 1) * DH]
        # bvb[p, s, plane, f] = bv_s[f]  (broadcast over partitions/planes)
        bvb = np.empty((128, 2, 2, 512), bf16)
        for s in range(2):
            bvs = np.asarray(inputs["bv" + str(s + 1)], f32)[hsl].astype(bf16)
            bvb[:, s] = np.broadcast_to(bvs, (128, 2, 512))

        in_maps.append({
            "xT": xT, "wq": wq, "wk": wk, "wv": wv, "wo": wo,
            "bias_qk": bias, "bvb": bvb, "cosT": cosT, "sinT": sinT, "Rt": rt,
        })
    return in_maps


def unshard_outputs(results, inputs):
    f32 = np.float32
    acc = np.zeros((B, D, T), f32)
    for c in range(N_CORES):
        # outT [128, NTT, NKC, 512] -> [D, T]: out[od*128+p, qt*512+j]
        r = results[c]["outT"].astype(f32)
        acc[c // 4] += r.transpose(2, 0, 1, 3).reshape(D, T)
    o1 = np.empty((B, N1, D), f32)
    o2 = np.empty((B, N2, D), f32)
    bo1 = np.asarray(inputs["bo1"], f32)
    bo2 = np.asarray(inputs["bo2"], f32)
    for b in range(B):
        full = acc[b].T                                  # [T, D]
        o1[b] = full[:N1] + bo1
        o2[b] = full[N1:] + bo2
    return o1, o2


def kernel(**inputs):
    nc = build_program()
    in_maps = shard_inputs(inputs)
    res = run_bass_kernel_spmd(nc, in_maps, list(range(N_CORES)))
    return unshard_outputs(res.results, inputs)


if __name__ == "__main__":
    data = np.load("/root/problem/cache_inputs.npz")
    out = kernel(**{k: data[k] for k in data.files})
    exp = np.load("/root/problem/cache_expected.npz")
    for i, o in enumerate(out):
        e = exp[f"o{i+1}"]
        d = np.abs(o - e).max()
        print(f"o{i+1}: absmax_err {d:.4e} rel {d / np.abs(e).max():.4e}")

